# revision 1
# baseline (speedup 1.0000x reference)
"""CornerPool block (conv/BN/cummax-pool residual block) on 8 Trainium2
NeuronCores, pure data-parallel over batch (1 sample per core).

Reference computation per sample (x: [256, 128, 128] f32):
    res    = BN(conv1x1(x, w_res))
    p1     = relu(BN(conv3x3(x, w_vpre)))        # 256 -> 64
    pool1  = reverse-cummax(p1, axis=H)          # TopPool
    p2     = relu(BN(conv3x3(x, w_hpre)))        # 256 -> 64
    pool2  = reverse-cummax(p2, axis=W)          # LeftPool
    merged = BN(conv3x3(pool1 + pool2, w_add))   # 64 -> 256
    out    = relu(res + merged)
    y      = relu(BN(conv3x3(out, w_post)))      # 256 -> 256

Kernel strategy (per core):
  * BN folded into conv weights/biases host-side; every conv is a
    sum-of-9-shifted-taps matmul accumulation in PSUM (channels on the
    partition dim, pixels on the free dim, N=512 = 4 image rows).
  * vpre+hpre convs fused into one matmul stream (same rhs windows,
    64+64 output channels fill the 128-wide stationary operand).
  * Pooling as in-place DVE tensor_max scans on the padded [128,130,130]
    conv-output buffer (p1 on partitions 0:64, p2 on 64:128).
  * The merged conv contracts over all 128 partitions with the 64-row
    weight block replicated, which computes conv(pool1 + pool2) without
    materializing the sum.
  * res 1x1 conv re-reads the phase-A x strips still live in the SBUF
    ring; accumulates into the same PSUM group as the merged conv.
  * out is bounced through DRAM in 4-row strips; the post conv streams
    it back with halo. All phases are emitted interleaved in reverse
    strip order so the Tile scheduler overlaps them into one wavefront.
  * All matmuls use float32r (full fp32 data, 1 cycle/row at N=512).
"""

import sys

import numpy as np

if "/opt/trn_rl_repo" not in sys.path:
    sys.path.insert(0, "/opt/trn_rl_repo")

EPS = 1e-5
C, M = 256, 64
B, H, W = 8, 128, 128
S = 4                      # output rows per strip
NS = H // S                # 32 strips
HP, WP = H + 2, W + 2      # padded spatial dims
N_CORES = 8

_CACHE = {}


def _patch_tile_drain():
    """This walrus build rejects >2 packed sync waits on the TileContext
    exit Drain. Split them into standalone wait_ge instructions."""
    import concourse.tile as tile
    from concourse.vector_clock import ScopedClock

    if getattr(tile.TileContext._drain_and_barrier, "_split_waits", False):
        return

    def _drain_and_barrier(self, tick_clock, wait_clock):
        nc = self.nc
        probe = nc.sync.nop(nofuse=True)
        wait_clock.add_sem_waits(
            probe.ins, ScopedClock({None: tick_clock.global_clock})
        )
        waits = list(probe.ins.sync_info.on_wait)
        if len(waits) > 1:
            probe.ins.sync_info.on_wait = waits[:1]
            sems_by_id = {s.num: s for s in wait_clock.sems.allocated().values()}
            for w in waits[1:]:
                nc.sync.wait_ge(sems_by_id[w.id], w.wait_value)
        nc.sync.drain()
        nc.all_engine_barrier()
        popped = nc._tile_sem_poison_stack.pop()
        assert popped is self._sem_poison
        nc.clear_and_free_semaphores(list(self.sems.allocated().values()))
        nc.all_engine_barrier()

    _drain_and_barrier._split_waits = True
    tile.TileContext._drain_and_barrier = _drain_and_barrier


TAPS = [(dy, dx) for dy in range(3) for dx in range(3)]


def _legalize_waits(nc, mybir):
    """This walrus build accepts at most ONE sync wait per instruction
    (any class). Split excess waits into single-wait NoOps emitted just
    before the instruction on the same engine sequencer."""
    for f in nc.m.functions:
        for bb in f.blocks:
            insts = bb.instructions
            out = []
            for inst in insts:
                si = inst.sync_info
                waits = list(si.on_wait) if si is not None else []
                if len(waits) > 1:
                    for j, w in enumerate(waits[:-1]):
                        noop = mybir.InstNoOp(
                            name=f"{inst.name}-ws{j}",
                            sync_info=mybir.SyncInfo(on_wait=[w], on_update=[]),
                            bass_nofuse=True,
                            engine=inst.engine,
                        )
                        nc.register_instruction(noop)
                        out.append(noop)
                    si.on_wait = waits[-1:]
                out.append(inst)
            insts[:] = out


def build_nc(debug_taps=False):
    import concourse.bass as bass
    import concourse.mybir as mybir
    import concourse.tile as tile

    _patch_tile_drain()
    f32 = mybir.dt.float32
    f32r = mybir.dt.float32r
    Relu = mybir.ActivationFunctionType.Relu

    nc = bass.Bass()
    x_d = nc.declare_dram_parameter("x_s", [C, H, WP], f32r, isOutput=False)
    # lhsT weight banks, laid out [k(part), idx, m]
    wvh_d = nc.declare_dram_parameter("w_vh", [128, 18, 128], f32r, isOutput=False)
    wres_d = nc.declare_dram_parameter("w_res_l", [128, 4, 128], f32r, isOutput=False)
    wmrg_d = nc.declare_dram_parameter("w_mrg", [128, 18, 128], f32r, isOutput=False)
    wpost_d = nc.declare_dram_parameter("w_post_l", [128, 36, 128], f32r, isOutput=False)
    bias_d = nc.declare_dram_parameter("biases", [128, 5], f32, isOutput=False)
    zeros_d = nc.declare_dram_parameter("zeros", [128, 4 * WP], f32r, isOutput=False)
    y_d = nc.declare_dram_parameter("y", [C, H, W], f32, isOutput=True)
    if debug_taps:
        dbg_pooled_d = nc.declare_dram_parameter(
            "dbg_pooled", [128, HP, WP], f32, isOutput=True)
        dbg_out_d = nc.declare_dram_parameter(
            "dbg_out", [2, 128, H, WP], f32, isOutput=True)

    with tile.TileContext(nc) as tc:
        with (
            tc.tile_pool(name="const", bufs=1) as constp,
            tc.tile_pool(name="big", bufs=1) as bigp,
            tc.tile_pool(name="stage", bufs=6) as stagep,
            tc.tile_pool(name="psum", bufs=8, space="PSUM") as psump,
            tc.tile_pool(name="dram", bufs=1, space="DRAM") as dramp,
        ):
            # DRAM bounce for `out` between the merge conv and the post
            # conv — a Tile-tracked DRAM tile so the strip DMAs get
            # read-after-write dependencies.
            outbuf_d = dramp.tile([2, 128, H, WP], f32r)
            # ---- constants on the phase-A critical path ----
            # Constants travel on the gpsimd SWDGE queues so they never
            # contend with the strip traffic on the 16 HWDGE queues.
            wvh = constp.tile([128, 18, 128], f32r)
            for j in range(0, 18, 3):
                nc.gpsimd.dma_start(wvh[:, j : j + 3, :], wvh_d[:, j : j + 3, :])
            bias = constp.tile([128, 5], f32)
            nc.gpsimd.dma_start(bias[:], bias_d[:])
            wres = constp.tile([128, 4, 128], f32r)
            wmrg = constp.tile([128, 18, 128], f32r)
            wpost = constp.tile([128, 36, 128], f32r)

            # ---- persistent buffers ----
            # conv-A output, padded; p1 on partitions 0:64, p2 on 64:128
            pooled = bigp.tile([128, HP, WP], f32r)

            def emit_deferred_consts():
                nc.gpsimd.dma_start(wres[:], wres_d[:])
                for j in range(0, 18, 5):
                    e = min(j + 5, 18)
                    nc.gpsimd.dma_start(wmrg[:, j:e, :], wmrg_d[:, j:e, :])
                for j in range(0, 36, 5):
                    e = min(j + 5, 36)
                    nc.gpsimd.dma_start(wpost[:, j:e, :], wpost_d[:, j:e, :])
                # Memset is not ISA-legal for f32r on this toolchain;
                # zero the conv pad regions via DMA from a zeros param.
                nc.gpsimd.dma_start(pooled[:, 0, :], zeros_d[:, :WP])
                nc.gpsimd.dma_start(pooled[:, HP - 1, :], zeros_d[:, :WP])
                nc.sync.dma_start(pooled[:, 1 : HP - 1, 0:1], zeros_d[:, : HP - 2])
                nc.sync.dma_start(pooled[:, 1 : HP - 1, WP - 1 : WP], zeros_d[:, : HP - 2])

            # x strip ring for phase A: 4 slots x 2 channel-tiles
            xbuf = [
                [bigp.tile([128, S + 2, WP], f32r, name=f"xbuf{j}_{kt}")
                 for kt in range(2)]
                for j in range(4)
            ]
            # C output staging ring: padded width, pad cols zeroed once so
            # the bounce DMAs stay contiguous end-to-end
            obuf = [bigp.tile([128, S, WP], f32r, name=f"obuf{j}")
                    for j in range(6)]
            for j in range(6):
                nc.sync.dma_start(obuf[j][:], zeros_d[:])

            # x strip ring for the res conv in phase C (full padded width
            # so the DMA stays contiguous; the matmul reads cols 1..128)
            cbuf = [
                [bigp.tile([128, S, WP], f32r, name=f"cbuf{j}_{kt}")
                 for kt in range(2)]
                for j in range(3)
            ]
            # out strip ring for the post conv: 3 slots x 2 channel-tiles
            dbuf = [
                [bigp.tile([128, S + 2, WP], f32r, name=f"dbuf{j}_{ct}")
                 for ct in range(2)]
                for j in range(3)
            ]

            def emit_A(s):
                """conv(x, [w_vpre|w_hpre]) + BN + relu for rows 4s..4s+3."""
                r = S * s
                xb = xbuf[s % 4]
                lo = max(0, r - 1)
                hi = min(H, r + S + 1)
                dst_lo = lo - (r - 1)
                for kt in range(2):
                    if s == 0:
                        # slot previously held a later strip's rows; row -1 pad
                        nc.sync.dma_start(xb[kt][:, 0, :], zeros_d[:, :WP])
                    elif s == NS - 1:
                        # first use of the slot: bottom halo row is pad
                        nc.sync.dma_start(xb[kt][:, S + 1, :], zeros_d[:, :WP])
                    nc.sync.dma_start(
                        xb[kt][:, dst_lo : dst_lo + (hi - lo), :],
                        x_d[kt * 128 : (kt + 1) * 128, lo:hi, :],
                    )
                ps = psump.tile([128, S * W], f32, tag="ps")
                n = len(TAPS) * 2
                i = 0
                for kt in range(2):
                    for t, (dy, dx) in enumerate(TAPS):
                        nc.tensor.matmul(
                            ps[:],
                            wvh[:, kt * 9 + t, :],
                            xb[kt][:, dy : dy + S, dx : dx + W],
                            start=(i == 0),
                            stop=(i == n - 1),
                        )
                        i += 1
                nc.scalar.activation(
                    pooled[:, r + 1 : r + 1 + S, 1 : 1 + W],
                    ps[:],
                    Relu,
                    bias=bias[:, 0:1],
                )

            def emit_toppool(s):
                r = S * s
                for y in range(min(H - 2, r + S - 1), r - 1, -1):
                    nc.vector.tensor_max(
                        pooled[0:64, y + 1, 1 : 1 + W],
                        pooled[0:64, y + 1, 1 : 1 + W],
                        pooled[0:64, y + 2, 1 : 1 + W],
                    )

            def emit_leftpool(s):
                # rows 4s .. 4s+31 (strips s..s+7 just completed)
                rlo, rhi = S * s + 1, S * s + 33
                for x in range(W - 2, -1, -1):
                    nc.vector.tensor_max(
                        pooled[64:128, rlo:rhi, x + 1],
                        pooled[64:128, rlo:rhi, x + 1],
                        pooled[64:128, rlo:rhi, x + 2],
                    )

            def emit_C(s):
                """res conv + merged conv + add + relu -> out_bounce strip."""
                r = S * s
                cb = cbuf[s % 3]
                for kt in range(2):
                    nc.sync.dma_start(
                        cb[kt][:],
                        x_d[kt * 128 : (kt + 1) * 128, r : r + S, :],
                    )
                for ct in range(2):
                    ps = psump.tile([128, S * W], f32, tag="ps")
                    for kt in range(2):
                        nc.tensor.matmul(
                            ps[:],
                            wres[:, ct * 2 + kt, :],
                            cb[kt][:, :, 1 : 1 + W],
                            start=(kt == 0),
                            stop=False,
                        )
                    for t, (dy, dx) in enumerate(TAPS):
                        nc.tensor.matmul(
                            ps[:],
                            wmrg[:, ct * 9 + t, :],
                            pooled[:, r + dy : r + dy + S, dx : dx + W],
                            start=False,
                            stop=(t == 8),
                        )
                    st = obuf[(2 * s + ct) % 6]
                    nc.scalar.activation(
                        st[:, :, 1 : 1 + W], ps[:], Relu,
                        bias=bias[:, 1 + ct : 2 + ct])
                    nc.sync.dma_start(outbuf_d[ct, :, r : r + S, :], st[:])

            def emit_D(s):
                """post conv + BN + relu -> y strip."""
                r = S * s
                db = dbuf[s % 3]
                lo = max(0, r - 1)
                hi = min(H, r + S + 1)
                dst_lo = lo - (r - 1)
                for ct in range(2):
                    if s == 0:
                        nc.sync.dma_start(db[ct][:, 0, :], zeros_d[:, :WP])
                    elif s == NS - 1:
                        nc.sync.dma_start(db[ct][:, S + 1, :], zeros_d[:, :WP])
                    nc.sync.dma_start(
                        db[ct][:, dst_lo : dst_lo + (hi - lo), :],
                        outbuf_d[ct, :, lo:hi, :],
                    )
                for co in range(2):
                    ps = psump.tile([128, S * W], f32, tag="ps")
                    i = 0
                    for kt in range(2):
                        for t, (dy, dx) in enumerate(TAPS):
                            nc.tensor.matmul(
                                ps[:],
                                wpost[:, co * 18 + kt * 9 + t, :],
                                db[kt][:, dy : dy + S, dx : dx + W],
                                start=(i == 0),
                                stop=(i == 17),
                            )
                            i += 1
                    st = stagep.tile([128, S * W], f32, tag="std")
                    nc.scalar.activation(st[:], ps[:], Relu, bias=bias[:, 3 + co : 4 + co])
                    nc.sync.dma_start(y_d[co * 128 : (co + 1) * 128, r : r + S, :], st[:])

            # Software-pipelined wavefront in groups of 8 strips,
            # processed bottom-up so the reverse-cummax chains unlock
            # consumers as early as possible. The C/D batches for group k
            # are emitted AFTER group k-1's conv-A strips: the PE then has
            # a full group of conv-A matmuls to chew on while the DVE
            # runs the 32-row LeftPool chunk the C batch is waiting for.
            def emit_group_A(k):
                for s in range(8 * k + 7, 8 * k - 1, -1):
                    emit_A(s)
                    emit_toppool(s)
                emit_leftpool(8 * k)

            def emit_group_CD(k):
                for s in range(min(NS - 1, 8 * k + 8), 8 * k, -1):
                    emit_C(s)
                d_hi = NS - 1 if k == 3 else 8 * k + 9
                for s in range(d_hi, 8 * k + 1, -1):
                    emit_D(s)

            emit_deferred_consts()
            emit_group_A(3)
            for k in range(3, -1, -1):
                if k > 0:
                    emit_group_A(k - 1)
                emit_group_CD(k)
            emit_C(0)
            emit_D(1)
            emit_D(0)
            if debug_taps:
                nc.sync.dma_start(dbg_pooled_d[:], pooled[:])
                nc.sync.dma_start(dbg_out_d[:], outbuf_d[:])

    _legalize_waits(nc, mybir)
    return nc


def _fold_bn(w, bn):
    """BN(conv(x, w)) == conv(x, w * s[co]) + t[co]."""
    g, b, m, v = bn[0], bn[1], bn[2], bn[3]
    s = g / np.sqrt(v + EPS)
    t = b - m * s
    return w * s[:, None, None, None], t


def _prep_inputs(x, w_res, bn_res, w_vpre, bn_vpre, w_hpre, bn_hpre,
                 w_add, bn_add, w_post, bn_post):
    x = np.asarray(x, np.float32)
    xp = np.zeros((B, C, H, WP), np.float32)
    xp[:, :, :, 1 : 1 + W] = x
    x = xp
    w_res_s, t_res = _fold_bn(np.asarray(w_res, np.float32), np.asarray(bn_res, np.float32))
    w_vpre_s, t_vpre = _fold_bn(np.asarray(w_vpre, np.float32), np.asarray(bn_vpre, np.float32))
    w_hpre_s, t_hpre = _fold_bn(np.asarray(w_hpre, np.float32), np.asarray(bn_hpre, np.float32))
    w_add_s, t_add = _fold_bn(np.asarray(w_add, np.float32), np.asarray(bn_add, np.float32))
    w_post_s, t_post = _fold_bn(np.asarray(w_post, np.float32), np.asarray(bn_post, np.float32))

    # w_vh[k, kt*9+t, m]: m<64 vpre, m>=64 hpre; lhsT[k, m] = w[m, kt*128+k, dy, dx]
    w_vh = np.zeros((128, 18, 128), np.float32)
    for kt in range(2):
        for t, (dy, dx) in enumerate(TAPS):
            blk = kt * 128
            w_vh[:, kt * 9 + t, 0:64] = w_vpre_s[:, blk : blk + 128, dy, dx].T
            w_vh[:, kt * 9 + t, 64:128] = w_hpre_s[:, blk : blk + 128, dy, dx].T

    # w_res_l[k, ct*2+kt, m] = w_res_s[ct*128+m, kt*128+k]
    w_res_l = np.zeros((128, 4, 128), np.float32)
    for ct in range(2):
        for kt in range(2):
            w_res_l[:, ct * 2 + kt, :] = w_res_s[
                ct * 128 : (ct + 1) * 128, kt * 128 : (kt + 1) * 128, 0, 0
            ].T

    # w_mrg[k, ct*9+t, m] = w_add_s[ct*128+m, k%64, dy, dx]  (row-replicated)
    w_mrg = np.zeros((128, 18, 128), np.float32)
    for ct in range(2):
        for t, (dy, dx) in enumerate(TAPS):
            blkT = w_add_s[ct * 128 : (ct + 1) * 128, :, dy, dx].T  # [64, 128]
            w_mrg[0:64, ct * 9 + t, :] = blkT
            w_mrg[64:128, ct * 9 + t, :] = blkT

    # w_post_l[k, co*18+kt*9+t, m] = w_post_s[co*128+m, kt*128+k, dy, dx]
    w_post_l = np.zeros((128, 36, 128), np.float32)
    for co in range(2):
        for kt in range(2):
            for t, (dy, dx) in enumerate(TAPS):
                w_post_l[:, co * 18 + kt * 9 + t, :] = w_post_s[
                    co * 128 : (co + 1) * 128, kt * 128 : (kt + 1) * 128, dy, dx
                ].T

    biases = np.zeros((128, 5), np.float32)
    biases[0:64, 0] = t_vpre
    biases[64:128, 0] = t_hpre
    t_mrg = t_res + t_add
    biases[:, 1] = t_mrg[0:128]
    biases[:, 2] = t_mrg[128:256]
    biases[:, 3] = t_post[0:128]
    biases[:, 4] = t_post[128:256]

    shared = {
        "zeros": np.zeros((128, 4 * WP), np.float32),
        "w_vh": w_vh,
        "w_res_l": w_res_l,
        "w_mrg": w_mrg,
        "w_post_l": w_post_l,
        "biases": biases,
    }
    return x, shared


def kernel(x, w_res, bn_res, w_vpre, bn_vpre, w_hpre, bn_hpre,
           w_add, bn_add, w_post, bn_post):
    from concourse.bass_utils import run_bass_kernel_spmd

    x, shared = _prep_inputs(x, w_res, bn_res, w_vpre, bn_vpre, w_hpre,
                             bn_hpre, w_add, bn_add, w_post, bn_post)

    if "nc" not in _CACHE:
        _CACHE["nc"] = build_nc()
    nc = _CACHE["nc"]

    in_maps = [dict(shared, x_s=np.ascontiguousarray(x[i])) for i in range(N_CORES)]
    res = run_bass_kernel_spmd(nc, in_maps, list(range(N_CORES)))
    return np.stack([res.results[i]["y"] for i in range(N_CORES)]).astype(np.float32)



# revision 8
# speedup vs baseline: 1.0752x; 1.0752x over previous
"""CornerPool block (conv/BN/cummax-pool residual block) on 8 Trainium2
NeuronCores, pure data-parallel over batch (1 sample per core).

Reference computation per sample (x: [256, 128, 128] f32):
    res    = BN(conv1x1(x, w_res))
    p1     = relu(BN(conv3x3(x, w_vpre)))        # 256 -> 64
    pool1  = reverse-cummax(p1, axis=H)          # TopPool
    p2     = relu(BN(conv3x3(x, w_hpre)))        # 256 -> 64
    pool2  = reverse-cummax(p2, axis=W)          # LeftPool
    merged = BN(conv3x3(pool1 + pool2, w_add))   # 64 -> 256
    out    = relu(res + merged)
    y      = relu(BN(conv3x3(out, w_post)))      # 256 -> 256

Kernel strategy (per core):
  * BN folded into conv weights/biases host-side; every conv is a
    sum-of-9-shifted-taps matmul accumulation in PSUM (channels on the
    partition dim, pixels on the free dim, N=512 = 4 image rows).
  * vpre+hpre convs fused into one matmul stream (same rhs windows,
    64+64 output channels fill the 128-wide stationary operand).
  * Pooling as in-place DVE tensor_max scans on the padded [128,130,130]
    conv-output buffer (p1 on partitions 0:64, p2 on 64:128).
  * The merged conv contracts over all 128 partitions with the 64-row
    weight block replicated, which computes conv(pool1 + pool2) without
    materializing the sum.
  * res 1x1 conv re-reads the phase-A x strips still live in the SBUF
    ring; accumulates into the same PSUM group as the merged conv.
  * out is bounced through DRAM in 4-row strips; the post conv streams
    it back with halo. All phases are emitted interleaved in reverse
    strip order so the Tile scheduler overlaps them into one wavefront.
  * All matmuls use float32r (full fp32 data, 1 cycle/row at N=512).
"""

import sys

import numpy as np

if "/opt/trn_rl_repo" not in sys.path:
    sys.path.insert(0, "/opt/trn_rl_repo")

EPS = 1e-5
C, M = 256, 64
B, H, W = 8, 128, 128
S = 4                      # output rows per strip
NS = H // S                # 32 strips
HP, WP = H + 2, W + 2      # padded spatial dims
N_CORES = 8

_CACHE = {}


def _patch_tile_drain():
    """This walrus build rejects >2 packed sync waits on the TileContext
    exit Drain. Split them into standalone wait_ge instructions."""
    import concourse.tile as tile
    from concourse.vector_clock import ScopedClock

    if getattr(tile.TileContext._drain_and_barrier, "_split_waits", False):
        return

    def _drain_and_barrier(self, tick_clock, wait_clock):
        nc = self.nc
        probe = nc.sync.nop(nofuse=True)
        wait_clock.add_sem_waits(
            probe.ins, ScopedClock({None: tick_clock.global_clock})
        )
        waits = list(probe.ins.sync_info.on_wait)
        if len(waits) > 1:
            probe.ins.sync_info.on_wait = waits[:1]
            sems_by_id = {s.num: s for s in wait_clock.sems.allocated().values()}
            for w in waits[1:]:
                nc.sync.wait_ge(sems_by_id[w.id], w.wait_value)
        nc.sync.drain()
        nc.all_engine_barrier()
        popped = nc._tile_sem_poison_stack.pop()
        assert popped is self._sem_poison
        nc.clear_and_free_semaphores(list(self.sems.allocated().values()))
        nc.all_engine_barrier()

    _drain_and_barrier._split_waits = True
    tile.TileContext._drain_and_barrier = _drain_and_barrier


TAPS = [(dy, dx) for dy in range(3) for dx in range(3)]


def _legalize_waits(nc, mybir):
    """This walrus build accepts at most ONE sync wait per instruction
    (any class). Split excess waits into single-wait NoOps emitted just
    before the instruction on the same engine sequencer."""
    for f in nc.m.functions:
        for bb in f.blocks:
            insts = bb.instructions
            out = []
            for inst in insts:
                si = inst.sync_info
                waits = list(si.on_wait) if si is not None else []
                if len(waits) > 1:
                    for j, w in enumerate(waits[:-1]):
                        noop = mybir.InstNoOp(
                            name=f"{inst.name}-ws{j}",
                            sync_info=mybir.SyncInfo(on_wait=[w], on_update=[]),
                            bass_nofuse=True,
                            engine=inst.engine,
                        )
                        nc.register_instruction(noop)
                        out.append(noop)
                    si.on_wait = waits[-1:]
                out.append(inst)
            insts[:] = out


def build_nc(debug_taps=False):
    import concourse.bass as bass
    import concourse.mybir as mybir
    import concourse.tile as tile

    _patch_tile_drain()
    f32 = mybir.dt.float32
    f32r = mybir.dt.float32r
    Relu = mybir.ActivationFunctionType.Relu

    nc = bass.Bass()
    x_d = nc.declare_dram_parameter("x_s", [C, H, WP], f32r, isOutput=False)
    # lhsT weight banks, laid out [k(part), idx, m]
    wvh_d = nc.declare_dram_parameter("w_vh", [128, 18, 128], f32r, isOutput=False)
    wres_d = nc.declare_dram_parameter("w_res_l", [128, 4, 128], f32r, isOutput=False)
    wmrg_d = nc.declare_dram_parameter("w_mrg", [128, 18, 128], f32r, isOutput=False)
    wpost_d = nc.declare_dram_parameter("w_post_l", [128, 36, 128], f32r, isOutput=False)
    bias_d = nc.declare_dram_parameter("biases", [128, 5], f32, isOutput=False)
    y_d = nc.declare_dram_parameter("y", [C, H, W], f32, isOutput=True)

    def zfill(engine, ap):
        # Memset is not ISA-legal for f32r; zero through an f32 view.
        engine.memset(ap.bitcast(f32), 0.0)
    if debug_taps:
        dbg_pooled_d = nc.declare_dram_parameter(
            "dbg_pooled", [128, HP, WP], f32, isOutput=True)
        dbg_out_d = nc.declare_dram_parameter(
            "dbg_out", [2, 128, H, WP], f32, isOutput=True)

    with tile.TileContext(nc) as tc:
        with (
            tc.tile_pool(name="const", bufs=1) as constp,
            tc.tile_pool(name="big", bufs=1) as bigp,
            tc.tile_pool(name="stage", bufs=6) as stagep,
            tc.tile_pool(name="psum", bufs=8, space="PSUM") as psump,
            tc.tile_pool(name="dram", bufs=1, space="DRAM") as dramp,
        ):
            # DRAM bounce for `out` between the merge conv and the post
            # conv — a Tile-tracked DRAM tile so the strip DMAs get
            # read-after-write dependencies.
            outbuf_d = dramp.tile([2, 128, H, WP], f32r)
            # ---- constants on the phase-A critical path ----
            # Constants travel on the gpsimd SWDGE queues so they never
            # contend with the strip traffic on the 16 HWDGE queues.
            wvh = constp.tile([128, 18, 128], f32r)
            for j in range(0, 18, 3):
                nc.gpsimd.dma_start(wvh[:, j : j + 3, :], wvh_d[:, j : j + 3, :])
            bias = constp.tile([128, 5], f32)
            nc.gpsimd.dma_start(bias[:], bias_d[:])
            wres = constp.tile([128, 4, 128], f32r)
            wmrg = constp.tile([128, 18, 128], f32r)
            wpost = constp.tile([128, 36, 128], f32r)

            # ---- persistent buffers ----
            # conv-A output, padded; p1 on partitions 0:64, p2 on 64:128
            pooled = bigp.tile([128, HP, WP], f32r)

            def emit_deferred_consts():
                nc.gpsimd.dma_start(wres[:], wres_d[:])
                for j in range(0, 18, 5):
                    e = min(j + 5, 18)
                    nc.gpsimd.dma_start(wmrg[:, j:e, :], wmrg_d[:, j:e, :])
                for j in range(0, 36, 5):
                    e = min(j + 5, 36)
                    nc.gpsimd.dma_start(wpost[:, j:e, :], wpost_d[:, j:e, :])
                # Zero the conv pad regions (engine memsets; keeps the DMA
                # queues free for the strip traffic on the critical path).
                zfill(nc.gpsimd, pooled[:, 0, :])
                zfill(nc.gpsimd, pooled[:, HP - 1, :])
                zfill(nc.gpsimd, pooled[:, 1 : HP - 1, 0:1])
                zfill(nc.gpsimd, pooled[:, 1 : HP - 1, WP - 1 : WP])

            # x strip ring for phase A: 4 slots x 2 channel-tiles
            xbuf = [
                [bigp.tile([128, S + 2, WP], f32r, name=f"xbuf{j}_{kt}")
                 for kt in range(2)]
                for j in range(4)
            ]
            # C output staging ring: padded width, pad cols zeroed once so
            # the bounce DMAs stay contiguous end-to-end
            obuf = [bigp.tile([128, S, WP], f32r, name=f"obuf{j}")
                    for j in range(6)]
            for j in range(6):
                zfill(nc.vector, obuf[j][:])

            # x strip ring for the res conv in phase C (full padded width
            # so the DMA stays contiguous; the matmul reads cols 1..128)
            cbuf = [
                [bigp.tile([128, S, WP], f32r, name=f"cbuf{j}_{kt}")
                 for kt in range(2)]
                for j in range(3)
            ]
            # out strip ring for the post conv: 3 slots x 2 channel-tiles
            dbuf = [
                [bigp.tile([128, S + 2, WP], f32r, name=f"dbuf{j}_{ct}")
                 for ct in range(2)]
                for j in range(3)
            ]

            def emit_A(s):
                """conv(x, [w_vpre|w_hpre]) + BN + relu for rows 4s..4s+3."""
                r = S * s
                xb = xbuf[s % 4]
                lo = max(0, r - 1)
                hi = min(H, r + S + 1)
                dst_lo = lo - (r - 1)
                for kt in range(2):
                    if s == 0:
                        # slot previously held a later strip's rows; row -1 pad
                        zfill(nc.vector, xb[kt][:, 0, :])
                    elif s == NS - 1:
                        # first use of the slot: bottom halo row is pad
                        zfill(nc.vector, xb[kt][:, S + 1, :])
                    nc.sync.dma_start(
                        xb[kt][:, dst_lo : dst_lo + (hi - lo), :],
                        x_d[kt * 128 : (kt + 1) * 128, lo:hi, :],
                    )
                ps = psump.tile([128, S * W], f32, tag="ps")
                n = len(TAPS) * 2
                i = 0
                for kt in range(2):
                    for t, (dy, dx) in enumerate(TAPS):
                        nc.tensor.matmul(
                            ps[:],
                            wvh[:, kt * 9 + t, :],
                            xb[kt][:, dy : dy + S, dx : dx + W],
                            start=(i == 0),
                            stop=(i == n - 1),
                        )
                        i += 1
                nc.scalar.activation(
                    pooled[:, r + 1 : r + 1 + S, 1 : 1 + W],
                    ps[:],
                    Relu,
                    bias=bias[:, 0:1],
                )

            def emit_toppool(s):
                r = S * s
                for y in range(min(H - 2, r + S - 1), r - 1, -1):
                    nc.vector.tensor_max(
                        pooled[0:64, y + 1, 1 : 1 + W],
                        pooled[0:64, y + 1, 1 : 1 + W],
                        pooled[0:64, y + 2, 1 : 1 + W],
                    )

            def emit_leftpool(s):
                # rows 4s .. 4s+31 (strips s..s+7 just completed).
                # Reverse cummax over the 128 data cols as a segmented scan
                # (8 segments x 16 cols): 15 within-segment steps vectorized
                # across segments + 7 segment-head steps + 1 broadcast step,
                # ~8us chain latency instead of 127 serial col steps.
                rlo, rhi = S * s + 1, S * s + 33
                V = pooled[64:128, rlo:rhi, 1:129].rearrange(
                    "p h (s i) -> p h s i", s=8)
                for i in range(14, -1, -1):
                    nc.vector.tensor_max(
                        V[:, :, :, i], V[:, :, :, i], V[:, :, :, i + 1])
                for sg in range(6, -1, -1):
                    nc.vector.tensor_max(
                        V[:, :, sg, 0:1], V[:, :, sg, 0:1],
                        V[:, :, sg + 1, 0:1])
                nc.vector.tensor_max(
                    V[:, :, 0:7, 1:16], V[:, :, 0:7, 1:16],
                    V[:, :, 1:8, 0:1].broadcast_to([64, 32, 7, 15]))

            def emit_C(s):
                """res conv + merged conv + add + relu -> out_bounce strip."""
                r = S * s
                cb = cbuf[s % 3]
                for kt in range(2):
                    nc.sync.dma_start(
                        cb[kt][:],
                        x_d[kt * 128 : (kt + 1) * 128, r : r + S, :],
                    )
                for ct in range(2):
                    ps = psump.tile([128, S * W], f32, tag="ps")
                    for kt in range(2):
                        nc.tensor.matmul(
                            ps[:],
                            wres[:, ct * 2 + kt, :],
                            cb[kt][:, :, 1 : 1 + W],
                            start=(kt == 0),
                            stop=False,
                        )
                    for t, (dy, dx) in enumerate(TAPS):
                        nc.tensor.matmul(
                            ps[:],
                            wmrg[:, ct * 9 + t, :],
                            pooled[:, r + dy : r + dy + S, dx : dx + W],
                            start=False,
                            stop=(t == 8),
                        )
                    st = obuf[(2 * s + ct) % 6]
                    nc.scalar.activation(
                        st[:, :, 1 : 1 + W], ps[:], Relu,
                        bias=bias[:, 1 + ct : 2 + ct])
                    nc.sync.dma_start(outbuf_d[ct, :, r : r + S, :], st[:])

            def emit_D(s):
                """post conv + BN + relu -> y strip."""
                r = S * s
                db = dbuf[s % 3]
                lo = max(0, r - 1)
                hi = min(H, r + S + 1)
                dst_lo = lo - (r - 1)
                for ct in range(2):
                    if s == 0:
                        zfill(nc.vector, db[ct][:, 0, :])
                    elif s == NS - 1:
                        zfill(nc.vector, db[ct][:, S + 1, :])
                    nc.sync.dma_start(
                        db[ct][:, dst_lo : dst_lo + (hi - lo), :],
                        outbuf_d[ct, :, lo:hi, :],
                    )
                for co in range(2):
                    ps = psump.tile([128, S * W], f32, tag="ps")
                    i = 0
                    for kt in range(2):
                        for t, (dy, dx) in enumerate(TAPS):
                            nc.tensor.matmul(
                                ps[:],
                                wpost[:, co * 18 + kt * 9 + t, :],
                                db[kt][:, dy : dy + S, dx : dx + W],
                                start=(i == 0),
                                stop=(i == 17),
                            )
                            i += 1
                    st = stagep.tile([128, S * W], f32, tag="std")
                    nc.scalar.activation(st[:], ps[:], Relu, bias=bias[:, 3 + co : 4 + co])
                    nc.sync.dma_start(y_d[co * 128 : (co + 1) * 128, r : r + S, :], st[:])

            # Software-pipelined wavefront in groups of 8 strips,
            # processed bottom-up so the reverse-cummax chains unlock
            # consumers as early as possible. The C/D batches for group k
            # are emitted AFTER group k-1's conv-A strips: the PE then has
            # a full group of conv-A matmuls to chew on while the DVE
            # runs the 32-row LeftPool chunk the C batch is waiting for.
            def emit_group_A(k):
                for s in range(8 * k + 7, 8 * k - 1, -1):
                    emit_A(s)
                    emit_toppool(s)
                emit_leftpool(8 * k)

            def emit_group_CD(k):
                for s in range(min(NS - 1, 8 * k + 8), 8 * k, -1):
                    emit_C(s)
                d_hi = NS - 1 if k == 3 else 8 * k + 9
                for s in range(d_hi, 8 * k + 1, -1):
                    emit_D(s)

            emit_deferred_consts()
            emit_group_A(3)
            for k in range(3, -1, -1):
                if k > 0:
                    emit_group_A(k - 1)
                emit_group_CD(k)
            emit_C(0)
            emit_D(1)
            emit_D(0)
            if debug_taps:
                nc.sync.dma_start(dbg_pooled_d[:], pooled[:])
                nc.sync.dma_start(dbg_out_d[:], outbuf_d[:])

    _legalize_waits(nc, mybir)
    return nc


def _fold_bn(w, bn):
    """BN(conv(x, w)) == conv(x, w * s[co]) + t[co]."""
    g, b, m, v = bn[0], bn[1], bn[2], bn[3]
    s = g / np.sqrt(v + EPS)
    t = b - m * s
    return w * s[:, None, None, None], t


def _prep_inputs(x, w_res, bn_res, w_vpre, bn_vpre, w_hpre, bn_hpre,
                 w_add, bn_add, w_post, bn_post):
    x = np.asarray(x, np.float32)
    xp = np.zeros((B, C, H, WP), np.float32)
    xp[:, :, :, 1 : 1 + W] = x
    x = xp
    w_res_s, t_res = _fold_bn(np.asarray(w_res, np.float32), np.asarray(bn_res, np.float32))
    w_vpre_s, t_vpre = _fold_bn(np.asarray(w_vpre, np.float32), np.asarray(bn_vpre, np.float32))
    w_hpre_s, t_hpre = _fold_bn(np.asarray(w_hpre, np.float32), np.asarray(bn_hpre, np.float32))
    w_add_s, t_add = _fold_bn(np.asarray(w_add, np.float32), np.asarray(bn_add, np.float32))
    w_post_s, t_post = _fold_bn(np.asarray(w_post, np.float32), np.asarray(bn_post, np.float32))

    # w_vh[k, kt*9+t, m]: m<64 vpre, m>=64 hpre; lhsT[k, m] = w[m, kt*128+k, dy, dx]
    w_vh = np.zeros((128, 18, 128), np.float32)
    for kt in range(2):
        for t, (dy, dx) in enumerate(TAPS):
            blk = kt * 128
            w_vh[:, kt * 9 + t, 0:64] = w_vpre_s[:, blk : blk + 128, dy, dx].T
            w_vh[:, kt * 9 + t, 64:128] = w_hpre_s[:, blk : blk + 128, dy, dx].T

    # w_res_l[k, ct*2+kt, m] = w_res_s[ct*128+m, kt*128+k]
    w_res_l = np.zeros((128, 4, 128), np.float32)
    for ct in range(2):
        for kt in range(2):
            w_res_l[:, ct * 2 + kt, :] = w_res_s[
                ct * 128 : (ct + 1) * 128, kt * 128 : (kt + 1) * 128, 0, 0
            ].T

    # w_mrg[k, ct*9+t, m] = w_add_s[ct*128+m, k%64, dy, dx]  (row-replicated)
    w_mrg = np.zeros((128, 18, 128), np.float32)
    for ct in range(2):
        for t, (dy, dx) in enumerate(TAPS):
            blkT = w_add_s[ct * 128 : (ct + 1) * 128, :, dy, dx].T  # [64, 128]
            w_mrg[0:64, ct * 9 + t, :] = blkT
            w_mrg[64:128, ct * 9 + t, :] = blkT

    # w_post_l[k, co*18+kt*9+t, m] = w_post_s[co*128+m, kt*128+k, dy, dx]
    w_post_l = np.zeros((128, 36, 128), np.float32)
    for co in range(2):
        for kt in range(2):
            for t, (dy, dx) in enumerate(TAPS):
                w_post_l[:, co * 18 + kt * 9 + t, :] = w_post_s[
                    co * 128 : (co + 1) * 128, kt * 128 : (kt + 1) * 128, dy, dx
                ].T

    biases = np.zeros((128, 5), np.float32)
    biases[0:64, 0] = t_vpre
    biases[64:128, 0] = t_hpre
    t_mrg = t_res + t_add
    biases[:, 1] = t_mrg[0:128]
    biases[:, 2] = t_mrg[128:256]
    biases[:, 3] = t_post[0:128]
    biases[:, 4] = t_post[128:256]

    shared = {
        "w_vh": w_vh,
        "w_res_l": w_res_l,
        "w_mrg": w_mrg,
        "w_post_l": w_post_l,
        "biases": biases,
    }
    return x, shared


def kernel(x, w_res, bn_res, w_vpre, bn_vpre, w_hpre, bn_hpre,
           w_add, bn_add, w_post, bn_post):
    from concourse.bass_utils import run_bass_kernel_spmd

    x, shared = _prep_inputs(x, w_res, bn_res, w_vpre, bn_vpre, w_hpre,
                             bn_hpre, w_add, bn_add, w_post, bn_post)

    if "nc" not in _CACHE:
        _CACHE["nc"] = build_nc()
    nc = _CACHE["nc"]

    in_maps = [dict(shared, x_s=np.ascontiguousarray(x[i])) for i in range(N_CORES)]
    res = run_bass_kernel_spmd(nc, in_maps, list(range(N_CORES)))
    return np.stack([res.results[i]["y"] for i in range(N_CORES)]).astype(np.float32)



# revision 20
# speedup vs baseline: 1.1964x; 1.1127x over previous
"""CornerPool block (conv/BN/cummax-pool residual block) on 8 Trainium2
NeuronCores, pure data-parallel over batch (1 sample per core).

Reference computation per sample (x: [256, 128, 128] f32):
    res    = BN(conv1x1(x, w_res))
    p1     = relu(BN(conv3x3(x, w_vpre)))        # 256 -> 64
    pool1  = reverse-cummax(p1, axis=H)          # TopPool
    p2     = relu(BN(conv3x3(x, w_hpre)))        # 256 -> 64
    pool2  = reverse-cummax(p2, axis=W)          # LeftPool
    merged = BN(conv3x3(pool1 + pool2, w_add))   # 64 -> 256
    out    = relu(res + merged)
    y      = relu(BN(conv3x3(out, w_post)))      # 256 -> 256

Kernel strategy (per core):
  * BN folded into conv weights/biases host-side; every conv is a
    sum-of-9-shifted-taps matmul accumulation in PSUM (channels on the
    partition dim, pixels on the free dim, N=512 = 4 image rows).
  * vpre+hpre convs fused into one matmul stream (same rhs windows,
    64+64 output channels fill the 128-wide stationary operand).
  * Pooling as in-place DVE tensor_max scans on the padded [128,130,130]
    conv-output buffer (p1 on partitions 0:64, p2 on 64:128).
  * The merged conv contracts over all 128 partitions with the 64-row
    weight block replicated, which computes conv(pool1 + pool2) without
    materializing the sum.
  * res 1x1 conv re-reads the phase-A x strips still live in the SBUF
    ring; accumulates into the same PSUM group as the merged conv.
  * out is bounced through DRAM in 4-row strips; the post conv streams
    it back with halo. All phases are emitted interleaved in reverse
    strip order so the Tile scheduler overlaps them into one wavefront.
  * All matmuls use float32r (full fp32 data, 1 cycle/row at N=512).
"""

import sys

import numpy as np

if "/opt/trn_rl_repo" not in sys.path:
    sys.path.insert(0, "/opt/trn_rl_repo")

EPS = 1e-5
C, M = 256, 64
B, H, W = 8, 128, 128
S = 4                      # output rows per strip
NS = H // S                # 32 strips
HP, WP = H + 2, W + 2      # padded spatial dims
N_CORES = 8

_CACHE = {}


def _patch_tile_drain():
    """This walrus build rejects >2 packed sync waits on the TileContext
    exit Drain. Split them into standalone wait_ge instructions."""
    import concourse.tile as tile
    from concourse.vector_clock import ScopedClock

    if getattr(tile.TileContext._drain_and_barrier, "_split_waits", False):
        return

    def _drain_and_barrier(self, tick_clock, wait_clock):
        nc = self.nc
        probe = nc.sync.nop(nofuse=True)
        wait_clock.add_sem_waits(
            probe.ins, ScopedClock({None: tick_clock.global_clock})
        )
        waits = list(probe.ins.sync_info.on_wait)
        if len(waits) > 1:
            probe.ins.sync_info.on_wait = waits[:1]
            sems_by_id = {s.num: s for s in wait_clock.sems.allocated().values()}
            for w in waits[1:]:
                nc.sync.wait_ge(sems_by_id[w.id], w.wait_value)
        nc.sync.drain()
        nc.all_engine_barrier()
        popped = nc._tile_sem_poison_stack.pop()
        assert popped is self._sem_poison
        nc.clear_and_free_semaphores(list(self.sems.allocated().values()))
        nc.all_engine_barrier()

    _drain_and_barrier._split_waits = True
    tile.TileContext._drain_and_barrier = _drain_and_barrier


TAPS = [(dy, dx) for dy in range(3) for dx in range(3)]


def _legalize_waits(nc, mybir):
    """This walrus build accepts at most ONE sync wait per instruction
    (any class). Split excess waits into single-wait NoOps emitted just
    before the instruction on the same engine sequencer."""
    for f in nc.m.functions:
        for bb in f.blocks:
            insts = bb.instructions
            out = []
            for inst in insts:
                si = inst.sync_info
                waits = list(si.on_wait) if si is not None else []
                if len(waits) > 1:
                    for j, w in enumerate(waits[:-1]):
                        noop = mybir.InstNoOp(
                            name=f"{inst.name}-ws{j}",
                            sync_info=mybir.SyncInfo(on_wait=[w], on_update=[]),
                            bass_nofuse=True,
                            engine=inst.engine,
                        )
                        nc.register_instruction(noop)
                        out.append(noop)
                    si.on_wait = waits[-1:]
                out.append(inst)
            insts[:] = out


def build_nc(debug_taps=False):
    import concourse.bass as bass
    import concourse.mybir as mybir
    import concourse.tile as tile

    _patch_tile_drain()
    f32 = mybir.dt.float32
    f32r = mybir.dt.float32r
    bf16 = mybir.dt.bfloat16
    Relu = mybir.ActivationFunctionType.Relu

    nc = bass.Bass()
    # Full bf16 data path: the PE rejects mixed 32/16-bit matmul inputs,
    # and bf16 weights halve the LDWEIGHTS issue time on the Tensor queue
    # (190ns -> ~95ns), which is what sets the 236ns/matmul cadence.
    # bf16 x/pooled/bounce also halve the strip DMA traffic and SBUF use.
    # Accumulation stays f32 in PSUM; ~0.7-1% rel err vs the 2e-2 gate.
    x_d = nc.declare_dram_parameter("x_s", [C, H, WP], bf16, isOutput=False)
    # lhsT weight banks, laid out [k(part), idx, m]
    wvh_d = nc.declare_dram_parameter("w_vh", [128, 18, 128], bf16, isOutput=False)
    wres_d = nc.declare_dram_parameter("w_res_l", [128, 4, 128], bf16, isOutput=False)
    wmrg_d = nc.declare_dram_parameter("w_mrg", [128, 18, 128], bf16, isOutput=False)
    wpost_d = nc.declare_dram_parameter("w_post_l", [128, 36, 128], bf16, isOutput=False)
    bias_d = nc.declare_dram_parameter("biases", [128, 5], f32, isOutput=False)
    y_d = nc.declare_dram_parameter("y", [C, H, W], f32, isOutput=True)

    def zfill(engine, ap):
        # Memset is not ISA-legal for f32r; zero those through an f32 view.
        if ap.dtype == f32r:
            ap = ap.bitcast(f32)
        engine.memset(ap, 0.0)
    if debug_taps:
        dbg_pooled_d = nc.declare_dram_parameter(
            "dbg_pooled", [128, HP, WP], f32, isOutput=True)
        dbg_out_d = nc.declare_dram_parameter(
            "dbg_out", [2, 128, H, WP], f32, isOutput=True)

    with tile.TileContext(nc) as tc:
        with (
            tc.tile_pool(name="const", bufs=1) as constp,
            tc.tile_pool(name="big", bufs=1) as bigp,
            tc.tile_pool(name="stage", bufs=6) as stagep,
            tc.tile_pool(name="psum", bufs=8, space="PSUM") as psump,
            tc.tile_pool(name="dram", bufs=1, space="DRAM") as dramp,
        ):
            # DRAM bounce for `out` between the merge conv and the post
            # conv — a Tile-tracked DRAM tile so the strip DMAs get
            # read-after-write dependencies.
            outbuf_d = dramp.tile([2, 128, H, WP], bf16)
            # ---- constants on the phase-A critical path ----
            # Constants travel on the gpsimd SWDGE queues so they never
            # contend with the strip traffic on the 16 HWDGE queues.
            wvh = constp.tile([128, 18, 128], bf16)
            for j in range(0, 18, 3):
                nc.gpsimd.dma_start(wvh[:, j : j + 3, :], wvh_d[:, j : j + 3, :])
            bias = constp.tile([128, 5], f32)
            nc.gpsimd.dma_start(bias[:], bias_d[:])
            wres = constp.tile([128, 4, 128], bf16)
            wmrg = constp.tile([128, 18, 128], bf16)
            wpost = constp.tile([128, 36, 128], bf16)

            # ---- persistent buffers ----
            # conv-A output, padded; p1 on partitions 0:64, p2 on 64:128
            pooled = bigp.tile([128, HP, WP], bf16)

            def emit_deferred_consts():
                nc.gpsimd.dma_start(wres[:], wres_d[:])
                for j in range(0, 18, 5):
                    e = min(j + 5, 18)
                    nc.gpsimd.dma_start(wmrg[:, j:e, :], wmrg_d[:, j:e, :])
                for j in range(0, 36, 5):
                    e = min(j + 5, 36)
                    nc.gpsimd.dma_start(wpost[:, j:e, :], wpost_d[:, j:e, :])
                # Zero the conv pad regions (engine memsets; keeps the DMA
                # queues free for the strip traffic on the critical path).
                zfill(nc.gpsimd, pooled[:, 0, :])
                zfill(nc.gpsimd, pooled[:, HP - 1, :])
                zfill(nc.gpsimd, pooled[:, 1 : HP - 1, 0:1])
                zfill(nc.gpsimd, pooled[:, 1 : HP - 1, WP - 1 : WP])

            # x strip ring for phase A: 4 slots x 2 channel-tiles
            xbuf = [
                [bigp.tile([128, S + 2, WP], bf16, name=f"xbuf{j}_{kt}")
                 for kt in range(2)]
                for j in range(4)
            ]
            # C output staging ring: padded width, pad cols zeroed once so
            # the bounce DMAs stay contiguous end-to-end
            obuf = [bigp.tile([128, S, WP], bf16, name=f"obuf{j}")
                    for j in range(6)]
            for j in range(6):
                zfill(nc.vector, obuf[j][:])

            # x strip ring for the res conv in phase C (full padded width
            # so the DMA stays contiguous; the matmul reads cols 1..128)
            cbuf = [
                [bigp.tile([128, S, WP], bf16, name=f"cbuf{j}_{kt}")
                 for kt in range(2)]
                for j in range(3)
            ]
            # out strip ring for the post conv: 3 slots x 2 channel-tiles
            dbuf = [
                [bigp.tile([128, S + 2, WP], bf16, name=f"dbuf{j}_{ct}")
                 for ct in range(2)]
                for j in range(3)
            ]

            def emit_A(s):
                """conv(x, [w_vpre|w_hpre]) + BN + relu for rows 4s..4s+3."""
                r = S * s
                xb = xbuf[s % 4]
                lo = max(0, r - 1)
                hi = min(H, r + S + 1)
                dst_lo = lo - (r - 1)
                for kt in range(2):
                    if s == 0:
                        # slot previously held a later strip's rows; row -1 pad
                        zfill(nc.vector, xb[kt][:, 0, :])
                    elif s == NS - 1:
                        # first use of the slot: bottom halo row is pad
                        zfill(nc.vector, xb[kt][:, S + 1, :])
                    # kt split across queues so back-to-back strip loads
                    # don't serialize on one DMA issue queue
                    eng = nc.sync if kt == 0 else nc.scalar
                    eng.dma_start(
                        xb[kt][:, dst_lo : dst_lo + (hi - lo), :],
                        x_d[kt * 128 : (kt + 1) * 128, lo:hi, :],
                    )
                ps = psump.tile([128, S * W], f32, tag="ps")
                n = len(TAPS) * 2
                i = 0
                for kt in range(2):
                    for t, (dy, dx) in enumerate(TAPS):
                        nc.tensor.matmul(
                            ps[:],
                            wvh[:, kt * 9 + t, :],
                            xb[kt][:, dy : dy + S, dx : dx + W],
                            start=(i == 0),
                            stop=(i == n - 1),
                        )
                        i += 1
                nc.scalar.activation(
                    pooled[:, r + 1 : r + 1 + S, 1 : 1 + W],
                    ps[:],
                    Relu,
                    bias=bias[:, 0:1],
                )

            def emit_toppool(s):
                r = S * s
                for y in range(min(H - 2, r + S - 1), r - 1, -1):
                    nc.vector.tensor_max(
                        pooled[0:64, y + 1, 1 : 1 + W],
                        pooled[0:64, y + 1, 1 : 1 + W],
                        pooled[0:64, y + 2, 1 : 1 + W],
                    )

            def emit_leftpool(s):
                # rows 4s .. 4s+31 (strips s..s+7 just completed).
                # Reverse cummax over the 128 data cols as a segmented scan
                # (8 segments x 16 cols): 15 within-segment steps vectorized
                # across segments + 7 segment-head steps + 1 broadcast step,
                # ~8us chain latency instead of 127 serial col steps.
                rlo, rhi = S * s + 1, S * s + 33
                V = pooled[64:128, rlo:rhi, 1:129].rearrange(
                    "p h (s i) -> p h s i", s=8)
                for i in range(14, -1, -1):
                    nc.vector.tensor_max(
                        V[:, :, :, i], V[:, :, :, i], V[:, :, :, i + 1])
                for sg in range(6, -1, -1):
                    nc.vector.tensor_max(
                        V[:, :, sg, 0:1], V[:, :, sg, 0:1],
                        V[:, :, sg + 1, 0:1])
                nc.vector.tensor_max(
                    V[:, :, 0:7, 1:16], V[:, :, 0:7, 1:16],
                    V[:, :, 1:8, 0:1].broadcast_to([64, 32, 7, 15]))

            def emit_C(s):
                """res conv + merged conv + add + relu -> out_bounce strip."""
                r = S * s
                cb = cbuf[s % 3]
                for kt in range(2):
                    eng = nc.sync if kt == 0 else nc.scalar
                    eng.dma_start(
                        cb[kt][:],
                        x_d[kt * 128 : (kt + 1) * 128, r : r + S, :],
                    )
                for ct in range(2):
                    ps = psump.tile([128, S * W], f32, tag="ps")
                    for kt in range(2):
                        nc.tensor.matmul(
                            ps[:],
                            wres[:, ct * 2 + kt, :],
                            cb[kt][:, :, 1 : 1 + W],
                            start=(kt == 0),
                            stop=False,
                        )
                    for t, (dy, dx) in enumerate(TAPS):
                        nc.tensor.matmul(
                            ps[:],
                            wmrg[:, ct * 9 + t, :],
                            pooled[:, r + dy : r + dy + S, dx : dx + W],
                            start=False,
                            stop=(t == 8),
                        )
                    st = obuf[(2 * s + ct) % 6]
                    nc.scalar.activation(
                        st[:, :, 1 : 1 + W], ps[:], Relu,
                        bias=bias[:, 1 + ct : 2 + ct])
                    nc.sync.dma_start(outbuf_d[ct, :, r : r + S, :], st[:])

            def emit_D(s):
                """post conv + BN + relu -> y strip."""
                r = S * s
                db = dbuf[s % 3]
                lo = max(0, r - 1)
                hi = min(H, r + S + 1)
                dst_lo = lo - (r - 1)
                for ct in range(2):
                    if s == 0:
                        zfill(nc.vector, db[ct][:, 0, :])
                    elif s == NS - 1:
                        zfill(nc.vector, db[ct][:, S + 1, :])
                    eng = nc.sync if ct == 0 else nc.scalar
                    eng.dma_start(
                        db[ct][:, dst_lo : dst_lo + (hi - lo), :],
                        outbuf_d[ct, :, lo:hi, :],
                    )
                for co in range(2):
                    ps = psump.tile([128, S * W], f32, tag="ps")
                    i = 0
                    for kt in range(2):
                        for t, (dy, dx) in enumerate(TAPS):
                            nc.tensor.matmul(
                                ps[:],
                                wpost[:, co * 18 + kt * 9 + t, :],
                                db[kt][:, dy : dy + S, dx : dx + W],
                                start=(i == 0),
                                stop=(i == 17),
                            )
                            i += 1
                    st = stagep.tile([128, S * W], f32, tag="std")
                    nc.scalar.activation(st[:], ps[:], Relu, bias=bias[:, 3 + co : 4 + co])
                    nc.sync.dma_start(y_d[co * 128 : (co + 1) * 128, r : r + S, :], st[:])

            # Software-pipelined wavefront in groups of 8 strips,
            # processed bottom-up so the reverse-cummax chains unlock
            # consumers as early as possible. The C/D batches for group k
            # are emitted AFTER group k-1's conv-A strips: the PE then has
            # a full group of conv-A matmuls to chew on while the DVE
            # runs the 32-row LeftPool chunk the C batch is waiting for.
            def emit_group_A(k):
                for s in range(8 * k + 7, 8 * k - 1, -1):
                    emit_A(s)
                    emit_toppool(s)
                emit_leftpool(8 * k)

            def emit_group_CD(k):
                # For the last group, fold C(0)/D(1)/D(0) into the batch so
                # the C(0)->bounce->D chain overlaps the D(9..2) matmuls
                # instead of serializing at the very end of the kernel.
                c_lo = -1 if k == 0 else 8 * k
                for s in range(min(NS - 1, 8 * k + 8), c_lo, -1):
                    emit_C(s)
                d_hi = NS - 1 if k == 3 else 8 * k + 9
                d_lo = -1 if k == 0 else 8 * k + 1
                for s in range(d_hi, d_lo, -1):
                    emit_D(s)

            emit_deferred_consts()
            emit_group_A(3)
            for k in range(3, -1, -1):
                if k > 0:
                    emit_group_A(k - 1)
                emit_group_CD(k)
            if debug_taps:
                nc.sync.dma_start(dbg_pooled_d[:], pooled[:])
                nc.sync.dma_start(dbg_out_d[:], outbuf_d[:])

    _legalize_waits(nc, mybir)
    return nc


def _fold_bn(w, bn):
    """BN(conv(x, w)) == conv(x, w * s[co]) + t[co]."""
    g, b, m, v = bn[0], bn[1], bn[2], bn[3]
    s = g / np.sqrt(v + EPS)
    t = b - m * s
    return w * s[:, None, None, None], t


def _prep_inputs(x, w_res, bn_res, w_vpre, bn_vpre, w_hpre, bn_hpre,
                 w_add, bn_add, w_post, bn_post):
    import ml_dtypes

    x = np.asarray(x, np.float32)
    xp = np.zeros((B, C, H, WP), np.float32)
    xp[:, :, :, 1 : 1 + W] = x
    x = xp.astype(ml_dtypes.bfloat16)
    w_res_s, t_res = _fold_bn(np.asarray(w_res, np.float32), np.asarray(bn_res, np.float32))
    w_vpre_s, t_vpre = _fold_bn(np.asarray(w_vpre, np.float32), np.asarray(bn_vpre, np.float32))
    w_hpre_s, t_hpre = _fold_bn(np.asarray(w_hpre, np.float32), np.asarray(bn_hpre, np.float32))
    w_add_s, t_add = _fold_bn(np.asarray(w_add, np.float32), np.asarray(bn_add, np.float32))
    w_post_s, t_post = _fold_bn(np.asarray(w_post, np.float32), np.asarray(bn_post, np.float32))

    # w_vh[k, kt*9+t, m]: m<64 vpre, m>=64 hpre; lhsT[k, m] = w[m, kt*128+k, dy, dx]
    w_vh = np.zeros((128, 18, 128), np.float32)
    for kt in range(2):
        for t, (dy, dx) in enumerate(TAPS):
            blk = kt * 128
            w_vh[:, kt * 9 + t, 0:64] = w_vpre_s[:, blk : blk + 128, dy, dx].T
            w_vh[:, kt * 9 + t, 64:128] = w_hpre_s[:, blk : blk + 128, dy, dx].T

    # w_res_l[k, ct*2+kt, m] = w_res_s[ct*128+m, kt*128+k]
    w_res_l = np.zeros((128, 4, 128), np.float32)
    for ct in range(2):
        for kt in range(2):
            w_res_l[:, ct * 2 + kt, :] = w_res_s[
                ct * 128 : (ct + 1) * 128, kt * 128 : (kt + 1) * 128, 0, 0
            ].T

    # w_mrg[k, ct*9+t, m] = w_add_s[ct*128+m, k%64, dy, dx]  (row-replicated)
    w_mrg = np.zeros((128, 18, 128), np.float32)
    for ct in range(2):
        for t, (dy, dx) in enumerate(TAPS):
            blkT = w_add_s[ct * 128 : (ct + 1) * 128, :, dy, dx].T  # [64, 128]
            w_mrg[0:64, ct * 9 + t, :] = blkT
            w_mrg[64:128, ct * 9 + t, :] = blkT

    # w_post_l[k, co*18+kt*9+t, m] = w_post_s[co*128+m, kt*128+k, dy, dx]
    w_post_l = np.zeros((128, 36, 128), np.float32)
    for co in range(2):
        for kt in range(2):
            for t, (dy, dx) in enumerate(TAPS):
                w_post_l[:, co * 18 + kt * 9 + t, :] = w_post_s[
                    co * 128 : (co + 1) * 128, kt * 128 : (kt + 1) * 128, dy, dx
                ].T

    biases = np.zeros((128, 5), np.float32)
    biases[0:64, 0] = t_vpre
    biases[64:128, 0] = t_hpre
    t_mrg = t_res + t_add
    biases[:, 1] = t_mrg[0:128]
    biases[:, 2] = t_mrg[128:256]
    biases[:, 3] = t_post[0:128]
    biases[:, 4] = t_post[128:256]

    shared = {
        "w_vh": w_vh.astype(ml_dtypes.bfloat16),
        "w_res_l": w_res_l.astype(ml_dtypes.bfloat16),
        "w_mrg": w_mrg.astype(ml_dtypes.bfloat16),
        "w_post_l": w_post_l.astype(ml_dtypes.bfloat16),
        "biases": biases,
    }
    return x, shared


def kernel(x, w_res, bn_res, w_vpre, bn_vpre, w_hpre, bn_hpre,
           w_add, bn_add, w_post, bn_post):
    from concourse.bass_utils import run_bass_kernel_spmd

    x, shared = _prep_inputs(x, w_res, bn_res, w_vpre, bn_vpre, w_hpre,
                             bn_hpre, w_add, bn_add, w_post, bn_post)

    if "nc" not in _CACHE:
        _CACHE["nc"] = build_nc()
    nc = _CACHE["nc"]

    in_maps = [dict(shared, x_s=np.ascontiguousarray(x[i])) for i in range(N_CORES)]
    res = run_bass_kernel_spmd(nc, in_maps, list(range(N_CORES)))
    return np.stack([res.results[i]["y"] for i in range(N_CORES)]).astype(np.float32)



# revision 33
# speedup vs baseline: 1.2202x; 1.0199x over previous
"""CornerPool block (conv/BN/cummax-pool residual block) on 8 Trainium2
NeuronCores, pure data-parallel over batch (1 sample per core).

Reference computation per sample (x: [256, 128, 128] f32):
    res    = BN(conv1x1(x, w_res))
    p1     = relu(BN(conv3x3(x, w_vpre)))        # 256 -> 64
    pool1  = reverse-cummax(p1, axis=H)          # TopPool
    p2     = relu(BN(conv3x3(x, w_hpre)))        # 256 -> 64
    pool2  = reverse-cummax(p2, axis=W)          # LeftPool
    merged = BN(conv3x3(pool1 + pool2, w_add))   # 64 -> 256
    out    = relu(res + merged)
    y      = relu(BN(conv3x3(out, w_post)))      # 256 -> 256

Kernel strategy (per core):
  * BN folded into conv weights/biases host-side; every conv is a
    sum-of-9-shifted-taps matmul accumulation in PSUM (channels on the
    partition dim, pixels on the free dim, N=512 = 4 image rows).
  * vpre+hpre convs fused into one matmul stream (same rhs windows,
    64+64 output channels fill the 128-wide stationary operand).
  * Pooling as in-place DVE tensor_max scans on the padded [128,130,130]
    conv-output buffer (p1 on partitions 0:64, p2 on 64:128).
  * The merged conv contracts over all 128 partitions with the 64-row
    weight block replicated, which computes conv(pool1 + pool2) without
    materializing the sum.
  * res 1x1 conv re-reads the phase-A x strips still live in the SBUF
    ring; accumulates into the same PSUM group as the merged conv.
  * out is bounced through DRAM in 4-row strips; the post conv streams
    it back with halo. All phases are emitted interleaved in reverse
    strip order so the Tile scheduler overlaps them into one wavefront.
  * All matmuls use float32r (full fp32 data, 1 cycle/row at N=512).
"""

import sys

import numpy as np

if "/opt/trn_rl_repo" not in sys.path:
    sys.path.insert(0, "/opt/trn_rl_repo")

EPS = 1e-5
C, M = 256, 64
B, H, W = 8, 128, 128
S = 4                      # output rows per strip
NS = H // S                # 32 strips
SS = 8                     # output rows per post-conv (Winograd) superstrip
NSS = H // SS              # 16 superstrips
NB = W // 2                # 64 Winograd F(2,3) column blocks
HP, WP = H + 2, W + 2      # padded spatial dims
N_CORES = 8

_CACHE = {}


def _patch_tile_drain():
    """This walrus build rejects >2 packed sync waits on the TileContext
    exit Drain. Split them into standalone wait_ge instructions."""
    import concourse.tile as tile
    from concourse.vector_clock import ScopedClock

    if getattr(tile.TileContext._drain_and_barrier, "_split_waits", False):
        return

    def _drain_and_barrier(self, tick_clock, wait_clock):
        nc = self.nc
        probe = nc.sync.nop(nofuse=True)
        wait_clock.add_sem_waits(
            probe.ins, ScopedClock({None: tick_clock.global_clock})
        )
        waits = list(probe.ins.sync_info.on_wait)
        if len(waits) > 1:
            probe.ins.sync_info.on_wait = waits[:1]
            sems_by_id = {s.num: s for s in wait_clock.sems.allocated().values()}
            for w in waits[1:]:
                nc.sync.wait_ge(sems_by_id[w.id], w.wait_value)
        nc.sync.drain()
        nc.all_engine_barrier()
        popped = nc._tile_sem_poison_stack.pop()
        assert popped is self._sem_poison
        nc.clear_and_free_semaphores(list(self.sems.allocated().values()))
        nc.all_engine_barrier()

    _drain_and_barrier._split_waits = True
    tile.TileContext._drain_and_barrier = _drain_and_barrier


TAPS = [(dy, dx) for dy in range(3) for dx in range(3)]


def _legalize_waits(nc, mybir):
    """This walrus build accepts at most ONE sync wait per instruction
    (any class). Split excess waits into single-wait NoOps emitted just
    before the instruction on the same engine sequencer."""
    for f in nc.m.functions:
        for bb in f.blocks:
            insts = bb.instructions
            out = []
            for inst in insts:
                si = inst.sync_info
                waits = list(si.on_wait) if si is not None else []
                if len(waits) > 1:
                    for j, w in enumerate(waits[:-1]):
                        noop = mybir.InstNoOp(
                            name=f"{inst.name}-ws{j}",
                            sync_info=mybir.SyncInfo(on_wait=[w], on_update=[]),
                            bass_nofuse=True,
                            engine=inst.engine,
                        )
                        nc.register_instruction(noop)
                        out.append(noop)
                    si.on_wait = waits[-1:]
                out.append(inst)
            insts[:] = out


def build_nc(debug_taps=False):
    import concourse.bass as bass
    import concourse.mybir as mybir
    import concourse.tile as tile

    _patch_tile_drain()
    f32 = mybir.dt.float32
    f32r = mybir.dt.float32r
    bf16 = mybir.dt.bfloat16
    Relu = mybir.ActivationFunctionType.Relu

    nc = bass.Bass()
    # Full bf16 data path: the PE rejects mixed 32/16-bit matmul inputs,
    # and bf16 weights halve the LDWEIGHTS issue time on the Tensor queue
    # (190ns -> ~95ns), which is what sets the 236ns/matmul cadence.
    # bf16 x/pooled/bounce also halve the strip DMA traffic and SBUF use.
    # Accumulation stays f32 in PSUM; ~0.7-1% rel err vs the 2e-2 gate.
    x_d = nc.declare_dram_parameter("x_s", [C, H, WP], bf16, isOutput=False)
    # lhsT weight banks, laid out [k(part), idx, m]
    wvh_d = nc.declare_dram_parameter("w_vh", [128, 18, 128], bf16, isOutput=False)
    wres_d = nc.declare_dram_parameter("w_res_l", [128, 4, 128], bf16, isOutput=False)
    wmrg_d = nc.declare_dram_parameter("w_mrg", [128, 18, 128], bf16, isOutput=False)
    wpost_d = nc.declare_dram_parameter("w_post_l", [128, 48, 128], bf16, isOutput=False)
    bias_d = nc.declare_dram_parameter("biases", [128, 5], f32, isOutput=False)
    y_d = nc.declare_dram_parameter("y", [C, H, W], f32, isOutput=True)

    def zfill(engine, ap):
        # Memset is not ISA-legal for f32r; zero those through an f32 view.
        if ap.dtype == f32r:
            ap = ap.bitcast(f32)
        engine.memset(ap, 0.0)
    if debug_taps:
        dbg_pooled_d = nc.declare_dram_parameter(
            "dbg_pooled", [128, HP, WP], f32, isOutput=True)
        dbg_out_d = nc.declare_dram_parameter(
            "dbg_out", [2, 128, HP, WP], f32, isOutput=True)

    with tile.TileContext(nc) as tc:
        with (
            tc.tile_pool(name="const", bufs=1) as constp,
            tc.tile_pool(name="big", bufs=1) as bigp,
            tc.tile_pool(name="stage", bufs=9) as stagep,
            # 4 single-bank tiles for the A/C convs + one 4-bank tile for
            # the Winograd post-conv point accumulators = all 8 PSUM banks
            tc.tile_pool(name="psum", bufs=4, space="PSUM") as psump,
            tc.tile_pool(name="wpsum", bufs=1, space="PSUM") as wpsump,
        ):
            # ---- constants on the phase-A critical path ----
            # Constants travel on the gpsimd SWDGE queues so they never
            # contend with the strip traffic on the 16 HWDGE queues.
            wvh = constp.tile([128, 18, 128], bf16)
            for j in range(0, 18, 3):
                nc.gpsimd.dma_start(wvh[:, j : j + 3, :], wvh_d[:, j : j + 3, :])
            bias = constp.tile([128, 5], f32)
            nc.gpsimd.dma_start(bias[:], bias_d[:])
            wres = constp.tile([128, 4, 128], bf16)
            wmrg = constp.tile([128, 18, 128], bf16)
            wpost = constp.tile([128, 48, 128], bf16)

            # ---- persistent buffers ----
            # conv-A output, padded; p1 on partitions 0:64, p2 on 64:128
            pooled = bigp.tile([128, HP, WP], bf16)

            def emit_deferred_consts():
                nc.gpsimd.dma_start(wres[:], wres_d[:])
                for j in range(0, 18, 5):
                    e = min(j + 5, 18)
                    nc.gpsimd.dma_start(wmrg[:, j:e, :], wmrg_d[:, j:e, :])
                for j in range(0, 48, 5):
                    e = min(j + 5, 48)
                    nc.gpsimd.dma_start(wpost[:, j:e, :], wpost_d[:, j:e, :])
                # Zero the conv pad regions (engine memsets; keeps the DMA
                # queues free for the strip traffic on the critical path).
                for buf in (pooled, outb[0], outb[1]):
                    zfill(nc.gpsimd, buf[:, 0, :])
                    zfill(nc.gpsimd, buf[:, HP - 1, :])
                    zfill(nc.gpsimd, buf[:, 1 : HP - 1, 0:1])
                    zfill(nc.gpsimd, buf[:, 1 : HP - 1, WP - 1 : WP])

            # x strip ring for phase A: 4 slots x 2 channel-tiles
            xbuf = [
                [bigp.tile([128, S + 2, WP], bf16, name=f"xbuf{j}_{kt}")
                 for kt in range(2)]
                for j in range(4)
            ]
            # `out` = relu(res + merged), SBUF-resident (bf16 makes it fit):
            # the C activations write strips straight into it and the
            # post-conv Winograd transform reads it in place — no DRAM
            # bounce. Padded like `pooled`; pad rows/cols zeroed once.
            outb = [bigp.tile([128, HP, WP], bf16, name=f"outb{ct}")
                    for ct in range(2)]

            # x strip ring for the res conv in phase C (full padded width
            # so the DMA stays contiguous; the matmul reads cols 1..128)
            cbuf = [
                [bigp.tile([128, S, WP], bf16, name=f"cbuf{j}_{kt}")
                 for kt in range(2)]
                for j in range(3)
            ]
            # Winograd F(2,3) V-tiles for the post conv: [p, halo row, block]
            vbuf = [
                [bigp.tile([128, 4, SS + 2, NB], bf16, name=f"vbuf{j}_{kt}")
                 for kt in range(2)]
                for j in range(2)
            ]
            # y output staging ring (f32, one superstrip x one co half)
            sty = [bigp.tile([128, SS, W], f32, name=f"sty{j}")
                   for j in range(3)]
            dctr = [0]

            def emit_A(s):
                """conv(x, [w_vpre|w_hpre]) + BN + relu for rows 4s..4s+3."""
                r = S * s
                xb = xbuf[s % 4]
                lo = max(0, r - 1)
                hi = min(H, r + S + 1)
                dst_lo = lo - (r - 1)
                for kt in range(2):
                    if s == 0:
                        # slot previously held a later strip's rows; row -1 pad
                        zfill(nc.vector, xb[kt][:, 0, :])
                    elif s == NS - 1:
                        # first use of the slot: bottom halo row is pad
                        zfill(nc.vector, xb[kt][:, S + 1, :])
                    # kt split across queues so back-to-back strip loads
                    # don't serialize on one DMA issue queue
                    eng = nc.sync if kt == 0 else nc.scalar
                    eng.dma_start(
                        xb[kt][:, dst_lo : dst_lo + (hi - lo), :],
                        x_d[kt * 128 : (kt + 1) * 128, lo:hi, :],
                    )
                ps = psump.tile([128, S * W], f32, tag="ps")
                n = len(TAPS) * 2
                i = 0
                for kt in range(2):
                    for t, (dy, dx) in enumerate(TAPS):
                        nc.tensor.matmul(
                            ps[:],
                            wvh[:, kt * 9 + t, :],
                            xb[kt][:, dy : dy + S, dx : dx + W],
                            start=(i == 0),
                            stop=(i == n - 1),
                        )
                        i += 1
                nc.scalar.activation(
                    pooled[:, r + 1 : r + 1 + S, 1 : 1 + W],
                    ps[:],
                    Relu,
                    bias=bias[:, 0:1],
                )

            def emit_toppool(s):
                r = S * s
                for y in range(min(H - 2, r + S - 1), r - 1, -1):
                    nc.vector.tensor_max(
                        pooled[0:64, y + 1, 1 : 1 + W],
                        pooled[0:64, y + 1, 1 : 1 + W],
                        pooled[0:64, y + 2, 1 : 1 + W],
                    )

            def emit_leftpool(s):
                # rows 4s .. 4s+31 (strips s..s+7 just completed).
                # Reverse cummax over the 128 data cols as a segmented scan
                # (8 segments x 16 cols): 15 within-segment steps vectorized
                # across segments + 7 segment-head steps + 1 broadcast step,
                # ~8us chain latency instead of 127 serial col steps.
                rlo, rhi = S * s + 1, S * s + 33
                V = pooled[64:128, rlo:rhi, 1:129].rearrange(
                    "p h (s i) -> p h s i", s=8)
                for i in range(14, -1, -1):
                    nc.vector.tensor_max(
                        V[:, :, :, i], V[:, :, :, i], V[:, :, :, i + 1])
                for sg in range(6, -1, -1):
                    nc.vector.tensor_max(
                        V[:, :, sg, 0:1], V[:, :, sg, 0:1],
                        V[:, :, sg + 1, 0:1])
                nc.vector.tensor_max(
                    V[:, :, 0:7, 1:16], V[:, :, 0:7, 1:16],
                    V[:, :, 1:8, 0:1].broadcast_to([64, 32, 7, 15]))

            def emit_C(s):
                """res conv + merged conv + add + relu -> out strip (SBUF)."""
                r = S * s
                cb = cbuf[s % 3]
                for kt in range(2):
                    eng = nc.sync if kt == 0 else nc.scalar
                    eng.dma_start(
                        cb[kt][:],
                        x_d[kt * 128 : (kt + 1) * 128, r : r + S, :],
                    )
                for ct in range(2):
                    ps = psump.tile([128, S * W], f32, tag="ps")
                    for kt in range(2):
                        nc.tensor.matmul(
                            ps[:],
                            wres[:, ct * 2 + kt, :],
                            cb[kt][:, :, 1 : 1 + W],
                            start=(kt == 0),
                            stop=False,
                        )
                    for t, (dy, dx) in enumerate(TAPS):
                        nc.tensor.matmul(
                            ps[:],
                            wmrg[:, ct * 9 + t, :],
                            pooled[:, r + dy : r + dy + S, dx : dx + W],
                            start=False,
                            stop=(t == 8),
                        )
                    nc.scalar.activation(
                        outb[ct][:, 1 + r : 1 + r + S, 1 : 1 + W],
                        ps[:].rearrange("p (r c) -> p r c", c=W),
                        Relu,
                        bias=bias[:, 1 + ct : 2 + ct])

            def emit_Dvt(ss):
                """1-D Winograd F(2,3) input transform of `out` rows
                8ss-1..8ss+8 (padded rows 8ss..8ss+9) into vbuf.
                Col blocks j pair output cols (2j, 2j+1); with padded cols
                d0..d3 = cols 2j..2j+3:  V0=d0-d2 V1=d1+d2 V2=d2-d1 V3=d1-d3."""
                R = SS * ss
                vb = vbuf[ss % 2]
                for kt in range(2):
                    d4 = outb[kt][:, R : R + SS + 2, :].rearrange(
                        "p r (b t) -> p r b t", t=2)
                    d0, d1 = d4[:, :, 0:NB, 0], d4[:, :, 0:NB, 1]
                    d2, d3 = d4[:, :, 1 : NB + 1, 0], d4[:, :, 1 : NB + 1, 1]
                    v = vb[kt]
                    nc.vector.tensor_sub(v[:, 0], d0, d2)
                    nc.vector.tensor_add(v[:, 1], d1, d2)
                    nc.vector.tensor_sub(v[:, 2], d2, d1)
                    nc.vector.tensor_sub(v[:, 3], d1, d3)

            def emit_Dunit(ss, co):
                """Winograd post conv, one co half of one 8-row superstrip:
                4 point-accumulators m_p (one PSUM bank each), 6 matmuls per
                point (3 dy x 2 kt) at N=512; then the inverse transform
                y(2j) = m0+m1+m2, y(2j+1) = m1-m2-m3 on the DVE."""
                R = SS * ss
                vb = vbuf[ss % 2]
                mt = wpsump.tile([128, 4, SS * NB], f32, tag="mw")
                for p in range(4):
                    i = 0
                    for dy in range(3):
                        for kt in range(2):
                            idx = ((co * 3 + dy) * 4 + p) * 2 + kt
                            nc.tensor.matmul(
                                mt[:, p, :],
                                wpost[:, idx, :],
                                vb[kt][:, p, dy : dy + SS, :],
                                start=(i == 0),
                                stop=(i == 5),
                            )
                            i += 1
                mv = mt[:].rearrange("p q (r b) -> p q r b", b=NB)
                st = sty[dctr[0] % 3]
                dctr[0] += 1
                stv = st[:].rearrange("p r (b t) -> p r b t", t=2)
                # DVE ops may read at most ONE input from PSUM: stage m1
                # through SBUF, then each op touches a single m bank.
                t0 = stagep.tile([128, SS, NB], f32, tag="tw")
                t1 = stagep.tile([128, SS, NB], f32, tag="tw")
                t2 = stagep.tile([128, SS, NB], f32, tag="tw")
                nc.vector.tensor_copy(t0[:], mv[:, 1])
                nc.vector.tensor_add(t1[:], t0[:], mv[:, 2])
                nc.vector.tensor_add(stv[:, :, :, 0], t1[:], mv[:, 0])
                nc.vector.tensor_sub(t2[:], t0[:], mv[:, 2])
                nc.vector.tensor_sub(stv[:, :, :, 1], t2[:], mv[:, 3])
                nc.scalar.activation(
                    st[:], st[:], Relu, bias=bias[:, 3 + co : 4 + co])
                nc.sync.dma_start(
                    y_d[co * 128 : (co + 1) * 128, R : R + SS, :], st[:])

            # Software-pipelined wavefront in groups of 8 strips,
            # processed bottom-up so the reverse-cummax chains unlock
            # consumers as early as possible. The C/D batches for group k
            # are emitted AFTER group k-1's conv-A strips: the PE then has
            # a full group of conv-A matmuls to chew on while the DVE
            # runs the 32-row LeftPool chunk the C batch is waiting for.
            def emit_group_A(k):
                for s in range(8 * k + 7, 8 * k - 1, -1):
                    emit_A(s)
                    emit_toppool(s)
                emit_leftpool(8 * k)

            def emit_group_CD(k):
                # C strips and Winograd-D superstrips interleaved: the
                # single 4-bank Winograd accumulator (wpsum, bufs=1) frees
                # only once the DVE inverse transform has read it, so a C
                # strip is placed between consecutive D units to keep the
                # PE fed during that read.  The V transform for ss is
                # emitted as soon as its last C dependency (strip 2ss-1,
                # i.e. `out` row 8ss-1) is out.
                cs = list(range(min(NS - 1, 8 * k + 8), (-1 if k == 0 else 8 * k), -1))
                sss = list(range(min(NSS - 1, 4 * k + 4), (-1 if k == 0 else 4 * k), -1))
                ci = 0

                def drain_c(until_leq):
                    nonlocal ci
                    while ci < len(cs) and cs[ci] > until_leq:
                        emit_C(cs[ci])
                        ci += 1

                for ss in sss:
                    drain_c(2 * ss - 2)
                    emit_Dvt(ss)
                    emit_Dunit(ss, 0)
                    if ci < len(cs):
                        emit_C(cs[ci])
                        ci += 1
                    emit_Dunit(ss, 1)
                drain_c(-1)

            emit_deferred_consts()
            emit_group_A(3)
            for k in range(3, -1, -1):
                if k > 0:
                    emit_group_A(k - 1)
                emit_group_CD(k)
            if debug_taps:
                nc.sync.dma_start(dbg_pooled_d[:], pooled[:])
                for ct in range(2):
                    nc.sync.dma_start(dbg_out_d[ct], outb[ct][:])

    _legalize_waits(nc, mybir)
    return nc


def _fold_bn(w, bn):
    """BN(conv(x, w)) == conv(x, w * s[co]) + t[co]."""
    g, b, m, v = bn[0], bn[1], bn[2], bn[3]
    s = g / np.sqrt(v + EPS)
    t = b - m * s
    return w * s[:, None, None, None], t


def _prep_inputs(x, w_res, bn_res, w_vpre, bn_vpre, w_hpre, bn_hpre,
                 w_add, bn_add, w_post, bn_post):
    import ml_dtypes

    x = np.asarray(x, np.float32)
    xp = np.zeros((B, C, H, WP), np.float32)
    xp[:, :, :, 1 : 1 + W] = x
    x = xp.astype(ml_dtypes.bfloat16)
    w_res_s, t_res = _fold_bn(np.asarray(w_res, np.float32), np.asarray(bn_res, np.float32))
    w_vpre_s, t_vpre = _fold_bn(np.asarray(w_vpre, np.float32), np.asarray(bn_vpre, np.float32))
    w_hpre_s, t_hpre = _fold_bn(np.asarray(w_hpre, np.float32), np.asarray(bn_hpre, np.float32))
    w_add_s, t_add = _fold_bn(np.asarray(w_add, np.float32), np.asarray(bn_add, np.float32))
    w_post_s, t_post = _fold_bn(np.asarray(w_post, np.float32), np.asarray(bn_post, np.float32))

    # w_vh[k, kt*9+t, m]: m<64 vpre, m>=64 hpre; lhsT[k, m] = w[m, kt*128+k, dy, dx]
    w_vh = np.zeros((128, 18, 128), np.float32)
    for kt in range(2):
        for t, (dy, dx) in enumerate(TAPS):
            blk = kt * 128
            w_vh[:, kt * 9 + t, 0:64] = w_vpre_s[:, blk : blk + 128, dy, dx].T
            w_vh[:, kt * 9 + t, 64:128] = w_hpre_s[:, blk : blk + 128, dy, dx].T

    # w_res_l[k, ct*2+kt, m] = w_res_s[ct*128+m, kt*128+k]
    w_res_l = np.zeros((128, 4, 128), np.float32)
    for ct in range(2):
        for kt in range(2):
            w_res_l[:, ct * 2 + kt, :] = w_res_s[
                ct * 128 : (ct + 1) * 128, kt * 128 : (kt + 1) * 128, 0, 0
            ].T

    # w_mrg[k, ct*9+t, m] = w_add_s[ct*128+m, k%64, dy, dx]  (row-replicated)
    w_mrg = np.zeros((128, 18, 128), np.float32)
    for ct in range(2):
        for t, (dy, dx) in enumerate(TAPS):
            blkT = w_add_s[ct * 128 : (ct + 1) * 128, :, dy, dx].T  # [64, 128]
            w_mrg[0:64, ct * 9 + t, :] = blkT
            w_mrg[64:128, ct * 9 + t, :] = blkT

    # Winograd F(2,3) post-conv weights: for each (co, dy, point p, kt),
    # g0=w0, g1=(w0+w1+w2)/2, g2=(w0-w1+w2)/2, g3=w2 over the dx taps.
    # w_post_l[k, ((co*3+dy)*4+p)*2+kt, m] = g_p[co*128+m, kt*128+k]
    w_post_l = np.zeros((128, 48, 128), np.float32)
    for co in range(2):
        for dy in range(3):
            W3 = w_post_s[co * 128 : (co + 1) * 128, :, dy, :]  # [m, cin, dx]
            g = [W3[:, :, 0],
                 0.5 * (W3[:, :, 0] + W3[:, :, 1] + W3[:, :, 2]),
                 0.5 * (W3[:, :, 0] - W3[:, :, 1] + W3[:, :, 2]),
                 W3[:, :, 2]]
            for p in range(4):
                for kt in range(2):
                    idx = ((co * 3 + dy) * 4 + p) * 2 + kt
                    w_post_l[:, idx, :] = g[p][:, kt * 128 : (kt + 1) * 128].T

    biases = np.zeros((128, 5), np.float32)
    biases[0:64, 0] = t_vpre
    biases[64:128, 0] = t_hpre
    t_mrg = t_res + t_add
    biases[:, 1] = t_mrg[0:128]
    biases[:, 2] = t_mrg[128:256]
    biases[:, 3] = t_post[0:128]
    biases[:, 4] = t_post[128:256]

    shared = {
        "w_vh": w_vh.astype(ml_dtypes.bfloat16),
        "w_res_l": w_res_l.astype(ml_dtypes.bfloat16),
        "w_mrg": w_mrg.astype(ml_dtypes.bfloat16),
        "w_post_l": w_post_l.astype(ml_dtypes.bfloat16),
        "biases": biases,
    }
    return x, shared


def kernel(x, w_res, bn_res, w_vpre, bn_vpre, w_hpre, bn_hpre,
           w_add, bn_add, w_post, bn_post):
    from concourse.bass_utils import run_bass_kernel_spmd

    x, shared = _prep_inputs(x, w_res, bn_res, w_vpre, bn_vpre, w_hpre,
                             bn_hpre, w_add, bn_add, w_post, bn_post)

    if "nc" not in _CACHE:
        _CACHE["nc"] = build_nc()
    nc = _CACHE["nc"]

    in_maps = [dict(shared, x_s=np.ascontiguousarray(x[i])) for i in range(N_CORES)]
    res = run_bass_kernel_spmd(nc, in_maps, list(range(N_CORES)))
    return np.stack([res.results[i]["y"] for i in range(N_CORES)]).astype(np.float32)



# revision 39
# speedup vs baseline: 1.2225x; 1.0019x over previous
"""CornerPool block (conv/BN/cummax-pool residual block) on 8 Trainium2
NeuronCores, pure data-parallel over batch (1 sample per core).

Reference computation per sample (x: [256, 128, 128] f32):
    res    = BN(conv1x1(x, w_res))
    p1     = relu(BN(conv3x3(x, w_vpre)))        # 256 -> 64
    pool1  = reverse-cummax(p1, axis=H)          # TopPool
    p2     = relu(BN(conv3x3(x, w_hpre)))        # 256 -> 64
    pool2  = reverse-cummax(p2, axis=W)          # LeftPool
    merged = BN(conv3x3(pool1 + pool2, w_add))   # 64 -> 256
    out    = relu(res + merged)
    y      = relu(BN(conv3x3(out, w_post)))      # 256 -> 256

Kernel strategy (per core):
  * BN folded into conv weights/biases host-side; convs are shifted-tap
    matmul accumulations in PSUM (channels on the partition dim, pixels
    on the free dim, N=512 rows streamed per matmul).
  * Full bf16 data path (f32 PSUM accumulation): bf16 LDWEIGHTS (~96ns)
    hides under the 512-row matmul (~215ns), so the PE issues matmuls
    back-to-back at the row-streaming floor.
  * vpre+hpre convs fused into one matmul stream (64+64 output channels
    fill the 128-wide stationary operand).
  * TopPool as in-place DVE tensor_max row scans; LeftPool as a
    segmented reverse-cummax scan (8 segments x 16 cols) on the GpSimd
    engine so its serial chain never blocks the DVE queue.
  * The merged conv contracts over all 128 partitions with the 64-row
    weight block replicated, computing conv(pool1 + pool2) without
    materializing the sum; the res 1x1 conv accumulates into the same
    PSUM group.
  * `out` = relu(res+merged) lives entirely in SBUF (bf16) — no DRAM
    bounce. The 256->256 post conv uses 1-D Winograd F(2,3) along W:
    4 point-matmuls per (dy, co) over transformed V tiles cut its
    matmul rows by 1/3; the 4 point accumulators occupy one 4-bank
    PSUM tile, and the inverse transform (DVE) + relu-bias activation
    produce y. C half-strips are interleaved between Winograd units so
    the PE stays fed while the accumulator drains.
  * All phases are emitted interleaved in reverse strip order so the
    Tile scheduler overlaps them into one wavefront.
"""

import sys

import numpy as np

if "/opt/trn_rl_repo" not in sys.path:
    sys.path.insert(0, "/opt/trn_rl_repo")

EPS = 1e-5
C, M = 256, 64
B, H, W = 8, 128, 128
S = 4                      # output rows per strip
NS = H // S                # 32 strips
SS = 8                     # output rows per post-conv (Winograd) superstrip
NSS = H // SS              # 16 superstrips
NB = W // 2                # 64 Winograd F(2,3) column blocks
HP, WP = H + 2, W + 2      # padded spatial dims
N_CORES = 8

_CACHE = {}


def _patch_tile_drain():
    """This walrus build rejects >2 packed sync waits on the TileContext
    exit Drain. Split them into standalone wait_ge instructions."""
    import concourse.tile as tile
    from concourse.vector_clock import ScopedClock

    if getattr(tile.TileContext._drain_and_barrier, "_split_waits", False):
        return

    def _drain_and_barrier(self, tick_clock, wait_clock):
        nc = self.nc
        probe = nc.sync.nop(nofuse=True)
        wait_clock.add_sem_waits(
            probe.ins, ScopedClock({None: tick_clock.global_clock})
        )
        waits = list(probe.ins.sync_info.on_wait)
        if len(waits) > 1:
            probe.ins.sync_info.on_wait = waits[:1]
            sems_by_id = {s.num: s for s in wait_clock.sems.allocated().values()}
            for w in waits[1:]:
                nc.sync.wait_ge(sems_by_id[w.id], w.wait_value)
        nc.sync.drain()
        nc.all_engine_barrier()
        popped = nc._tile_sem_poison_stack.pop()
        assert popped is self._sem_poison
        nc.clear_and_free_semaphores(list(self.sems.allocated().values()))
        nc.all_engine_barrier()

    _drain_and_barrier._split_waits = True
    tile.TileContext._drain_and_barrier = _drain_and_barrier


TAPS = [(dy, dx) for dy in range(3) for dx in range(3)]


def _legalize_waits(nc, mybir):
    """This walrus build accepts at most ONE sync wait per instruction
    (any class). Split excess waits into single-wait NoOps emitted just
    before the instruction on the same engine sequencer."""
    for f in nc.m.functions:
        for bb in f.blocks:
            insts = bb.instructions
            out = []
            for inst in insts:
                si = inst.sync_info
                waits = list(si.on_wait) if si is not None else []
                if len(waits) > 1:
                    for j, w in enumerate(waits[:-1]):
                        noop = mybir.InstNoOp(
                            name=f"{inst.name}-ws{j}",
                            sync_info=mybir.SyncInfo(on_wait=[w], on_update=[]),
                            bass_nofuse=True,
                            engine=inst.engine,
                        )
                        nc.register_instruction(noop)
                        out.append(noop)
                    si.on_wait = waits[-1:]
                out.append(inst)
            insts[:] = out


def build_nc(debug_taps=False):
    import concourse.bass as bass
    import concourse.mybir as mybir
    import concourse.tile as tile

    _patch_tile_drain()
    f32 = mybir.dt.float32
    f32r = mybir.dt.float32r
    bf16 = mybir.dt.bfloat16
    Relu = mybir.ActivationFunctionType.Relu

    nc = bass.Bass()
    # Full bf16 data path: the PE rejects mixed 32/16-bit matmul inputs,
    # and bf16 weights halve the LDWEIGHTS issue time on the Tensor queue
    # (190ns -> ~95ns), which is what sets the 236ns/matmul cadence.
    # bf16 x/pooled/bounce also halve the strip DMA traffic and SBUF use.
    # Accumulation stays f32 in PSUM; ~0.7-1% rel err vs the 2e-2 gate.
    x_d = nc.declare_dram_parameter("x_s", [C, H, WP], bf16, isOutput=False)
    # lhsT weight banks, laid out [k(part), idx, m]
    wvh_d = nc.declare_dram_parameter("w_vh", [128, 18, 128], bf16, isOutput=False)
    wres_d = nc.declare_dram_parameter("w_res_l", [128, 4, 128], bf16, isOutput=False)
    wmrg_d = nc.declare_dram_parameter("w_mrg", [128, 18, 128], bf16, isOutput=False)
    wpost_d = nc.declare_dram_parameter("w_post_l", [128, 48, 128], bf16, isOutput=False)
    bias_d = nc.declare_dram_parameter("biases", [128, 5], f32, isOutput=False)
    y_d = nc.declare_dram_parameter("y", [C, H, W], f32, isOutput=True)

    def zfill(engine, ap):
        # Memset is not ISA-legal for f32r; zero those through an f32 view.
        if ap.dtype == f32r:
            ap = ap.bitcast(f32)
        engine.memset(ap, 0.0)
    if debug_taps:
        dbg_pooled_d = nc.declare_dram_parameter(
            "dbg_pooled", [128, HP, WP], f32, isOutput=True)
        dbg_out_d = nc.declare_dram_parameter(
            "dbg_out", [2, 128, HP, WP], f32, isOutput=True)

    with tile.TileContext(nc) as tc:
        with (
            tc.tile_pool(name="const", bufs=1) as constp,
            tc.tile_pool(name="big", bufs=1) as bigp,
            tc.tile_pool(name="stage", bufs=9) as stagep,
            # 4 single-bank tiles for the A/C convs + one 4-bank tile for
            # the Winograd post-conv point accumulators = all 8 PSUM banks
            tc.tile_pool(name="psum", bufs=4, space="PSUM") as psump,
            tc.tile_pool(name="wpsum", bufs=1, space="PSUM") as wpsump,
        ):
            # ---- constants on the phase-A critical path ----
            # Constants travel on the gpsimd SWDGE queues so they never
            # contend with the strip traffic on the 16 HWDGE queues.
            wvh = constp.tile([128, 18, 128], bf16)
            for j in range(0, 18, 3):
                nc.gpsimd.dma_start(wvh[:, j : j + 3, :], wvh_d[:, j : j + 3, :])
            bias = constp.tile([128, 5], f32)
            nc.gpsimd.dma_start(bias[:], bias_d[:])
            wres = constp.tile([128, 4, 128], bf16)
            wmrg = constp.tile([128, 18, 128], bf16)
            wpost = constp.tile([128, 48, 128], bf16)

            # ---- persistent buffers ----
            # conv-A output, padded; p1 on partitions 0:64, p2 on 64:128
            pooled = bigp.tile([128, HP, WP], bf16)

            def emit_deferred_consts():
                nc.gpsimd.dma_start(wres[:], wres_d[:])
                for j in range(0, 18, 5):
                    e = min(j + 5, 18)
                    nc.gpsimd.dma_start(wmrg[:, j:e, :], wmrg_d[:, j:e, :])
                for j in range(0, 48, 5):
                    e = min(j + 5, 48)
                    nc.gpsimd.dma_start(wpost[:, j:e, :], wpost_d[:, j:e, :])
                # Zero the conv pad regions (engine memsets; keeps the DMA
                # queues free for the strip traffic on the critical path).
                for buf in (pooled, outb[0], outb[1]):
                    zfill(nc.gpsimd, buf[:, 0, :])
                    zfill(nc.gpsimd, buf[:, HP - 1, :])
                    zfill(nc.gpsimd, buf[:, 1 : HP - 1, 0:1])
                    zfill(nc.gpsimd, buf[:, 1 : HP - 1, WP - 1 : WP])

            # x strip ring for phase A: 4 slots x 2 channel-tiles
            xbuf = [
                [bigp.tile([128, S + 2, WP], bf16, name=f"xbuf{j}_{kt}")
                 for kt in range(2)]
                for j in range(4)
            ]
            # `out` = relu(res + merged), SBUF-resident (bf16 makes it fit):
            # the C activations write strips straight into it and the
            # post-conv Winograd transform reads it in place — no DRAM
            # bounce. Padded like `pooled`; pad rows/cols zeroed once.
            outb = [bigp.tile([128, HP, WP], bf16, name=f"outb{ct}")
                    for ct in range(2)]

            # x strip ring for the res conv in phase C (full padded width
            # so the DMA stays contiguous; the matmul reads cols 1..128)
            cbuf = [
                [bigp.tile([128, S, WP], bf16, name=f"cbuf{j}_{kt}")
                 for kt in range(2)]
                for j in range(3)
            ]
            # Winograd F(2,3) V-tiles for the post conv: [p, halo row, block]
            vbuf = [
                [bigp.tile([128, 4, SS + 2, NB], bf16, name=f"vbuf{j}_{kt}")
                 for kt in range(2)]
                for j in range(2)
            ]
            # y output staging ring (f32, one superstrip x one co half)
            sty = [bigp.tile([128, SS, W], f32, name=f"sty{j}")
                   for j in range(3)]
            dctr = [0]

            def emit_A(s):
                """conv(x, [w_vpre|w_hpre]) + BN + relu for rows 4s..4s+3."""
                r = S * s
                xb = xbuf[s % 4]
                lo = max(0, r - 1)
                hi = min(H, r + S + 1)
                dst_lo = lo - (r - 1)
                for kt in range(2):
                    if s == 0:
                        # slot previously held a later strip's rows; row -1 pad
                        zfill(nc.vector, xb[kt][:, 0, :])
                    elif s == NS - 1:
                        # first use of the slot: bottom halo row is pad
                        zfill(nc.vector, xb[kt][:, S + 1, :])
                    # kt split across queues so back-to-back strip loads
                    # don't serialize on one DMA issue queue
                    eng = nc.sync if kt == 0 else nc.scalar
                    eng.dma_start(
                        xb[kt][:, dst_lo : dst_lo + (hi - lo), :],
                        x_d[kt * 128 : (kt + 1) * 128, lo:hi, :],
                    )
                ps = psump.tile([128, S * W], f32, tag="ps")
                n = len(TAPS) * 2
                i = 0
                for kt in range(2):
                    for t, (dy, dx) in enumerate(TAPS):
                        nc.tensor.matmul(
                            ps[:],
                            wvh[:, kt * 9 + t, :],
                            xb[kt][:, dy : dy + S, dx : dx + W],
                            start=(i == 0),
                            stop=(i == n - 1),
                        )
                        i += 1
                nc.scalar.activation(
                    pooled[:, r + 1 : r + 1 + S, 1 : 1 + W],
                    ps[:],
                    Relu,
                    bias=bias[:, 0:1],
                )

            def emit_toppool(s):
                r = S * s
                for y in range(min(H - 2, r + S - 1), r - 1, -1):
                    nc.vector.tensor_max(
                        pooled[0:64, y + 1, 1 : 1 + W],
                        pooled[0:64, y + 1, 1 : 1 + W],
                        pooled[0:64, y + 2, 1 : 1 + W],
                    )

            def emit_leftpool(s):
                # rows 4s .. 4s+31 (strips s..s+7 just completed).
                # Reverse cummax over the 128 data cols as a segmented scan
                # (8 segments x 16 cols): 15 within-segment steps vectorized
                # across segments + 7 segment-head steps + 1 broadcast step,
                # ~8us chain latency instead of 127 serial col steps.
                rlo, rhi = S * s + 1, S * s + 33
                V = pooled[64:128, rlo:rhi, 1:129].rearrange(
                    "p h (s i) -> p h s i", s=8)
                for i in range(14, -1, -1):
                    nc.vector.tensor_max(
                        V[:, :, :, i], V[:, :, :, i], V[:, :, :, i + 1])
                for sg in range(6, -1, -1):
                    nc.vector.tensor_max(
                        V[:, :, sg, 0:1], V[:, :, sg, 0:1],
                        V[:, :, sg + 1, 0:1])
                nc.vector.tensor_max(
                    V[:, :, 0:7, 1:16], V[:, :, 0:7, 1:16],
                    V[:, :, 1:8, 0:1].broadcast_to([64, 32, 7, 15]))

            def emit_C(s, ct):
                """res conv + merged conv + add + relu -> out strip (SBUF).
                Emitted per co half: each half is an independent ~2.4us PE
                unit, used as a separator between Winograd D units."""
                r = S * s
                cb = cbuf[s % 3]
                if ct == 0:
                    for kt in range(2):
                        eng = nc.sync if kt == 0 else nc.scalar
                        eng.dma_start(
                            cb[kt][:],
                            x_d[kt * 128 : (kt + 1) * 128, r : r + S, :],
                        )
                ps = psump.tile([128, S * W], f32, tag="ps")
                for kt in range(2):
                    nc.tensor.matmul(
                        ps[:],
                        wres[:, ct * 2 + kt, :],
                        cb[kt][:, :, 1 : 1 + W],
                        start=(kt == 0),
                        stop=False,
                    )
                for t, (dy, dx) in enumerate(TAPS):
                    nc.tensor.matmul(
                        ps[:],
                        wmrg[:, ct * 9 + t, :],
                        pooled[:, r + dy : r + dy + S, dx : dx + W],
                        start=False,
                        stop=(t == 8),
                    )
                nc.scalar.activation(
                    outb[ct][:, 1 + r : 1 + r + S, 1 : 1 + W],
                    ps[:].rearrange("p (r c) -> p r c", c=W),
                    Relu,
                    bias=bias[:, 1 + ct : 2 + ct])

            def emit_Dvt(ss):
                """1-D Winograd F(2,3) input transform of `out` rows
                8ss-1..8ss+8 (padded rows 8ss..8ss+9) into vbuf.
                Col blocks j pair output cols (2j, 2j+1); with padded cols
                d0..d3 = cols 2j..2j+3:  V0=d0-d2 V1=d1+d2 V2=d2-d1 V3=d1-d3."""
                R = SS * ss
                vb = vbuf[ss % 2]
                for kt in range(2):
                    d4 = outb[kt][:, R : R + SS + 2, :].rearrange(
                        "p r (b t) -> p r b t", t=2)
                    d0, d1 = d4[:, :, 0:NB, 0], d4[:, :, 0:NB, 1]
                    d2, d3 = d4[:, :, 1 : NB + 1, 0], d4[:, :, 1 : NB + 1, 1]
                    v = vb[kt]
                    nc.vector.tensor_sub(v[:, 0], d0, d2)
                    nc.vector.tensor_add(v[:, 1], d1, d2)
                    nc.vector.tensor_sub(v[:, 2], d2, d1)
                    nc.vector.tensor_sub(v[:, 3], d1, d3)

            def emit_Dunit(ss, co):
                """Winograd post conv, one co half of one 8-row superstrip:
                4 point-accumulators m_p (one PSUM bank each), 6 matmuls per
                point (3 dy x 2 kt) at N=512; then the inverse transform
                y(2j) = m0+m1+m2, y(2j+1) = m1-m2-m3 on the DVE."""
                R = SS * ss
                vb = vbuf[ss % 2]
                mt = wpsump.tile([128, 4, SS * NB], f32, tag="mw")
                for p in range(4):
                    i = 0
                    for dy in range(3):
                        for kt in range(2):
                            idx = ((co * 3 + dy) * 4 + p) * 2 + kt
                            nc.tensor.matmul(
                                mt[:, p, :],
                                wpost[:, idx, :],
                                vb[kt][:, p, dy : dy + SS, :],
                                start=(i == 0),
                                stop=(i == 5),
                            )
                            i += 1
                mv = mt[:].rearrange("p q (r b) -> p q r b", b=NB)
                st = sty[dctr[0] % 3]
                dctr[0] += 1
                stv = st[:].rearrange("p r (b t) -> p r b t", t=2)
                # DVE ops may read at most ONE input from PSUM: stage m1
                # through SBUF, then each op touches a single m bank.
                t0 = stagep.tile([128, SS, NB], f32, tag="tw")
                t1 = stagep.tile([128, SS, NB], f32, tag="tw")
                t2 = stagep.tile([128, SS, NB], f32, tag="tw")
                nc.vector.tensor_copy(t0[:], mv[:, 1])
                nc.vector.tensor_add(t1[:], t0[:], mv[:, 2])
                nc.vector.tensor_add(stv[:, :, :, 0], t1[:], mv[:, 0])
                nc.vector.tensor_sub(t2[:], t0[:], mv[:, 2])
                nc.vector.tensor_sub(stv[:, :, :, 1], t2[:], mv[:, 3])
                nc.scalar.activation(
                    st[:], st[:], Relu, bias=bias[:, 3 + co : 4 + co])
                nc.sync.dma_start(
                    y_d[co * 128 : (co + 1) * 128, R : R + SS, :], st[:])

            # Software-pipelined wavefront in groups of 8 strips,
            # processed bottom-up so the reverse-cummax chains unlock
            # consumers as early as possible. The C/D batches for group k
            # are emitted AFTER group k-1's conv-A strips: the PE then has
            # a full group of conv-A matmuls to chew on while the DVE
            # runs the 32-row LeftPool chunk the C batch is waiting for.
            def emit_group_A(k):
                for s in range(8 * k + 7, 8 * k - 1, -1):
                    emit_A(s)
                    emit_toppool(s)
                emit_leftpool(8 * k)

            def emit_group_CD(k):
                # C half-strips and Winograd-D superstrip units interleaved:
                # the single 4-bank Winograd accumulator (wpsum, bufs=1)
                # frees only once the DVE inverse transform has read it, so
                # a C half goes between consecutive D units to keep the PE
                # fed during that read.  C strips are shifted down one group
                # (cs = 8k+6..8k-1) so the top superstrip's V transform can
                # be pre-emitted at the END of the previous CD group, where
                # its inputs are already written — it then runs in the DVE
                # shadow of the next A group instead of stalling this one.
                cs = [(s, ct)
                      for s in range(NS - 1 if k == 3 else 8 * k + 6,
                                     (-1 if k == 0 else 8 * k - 2), -1)
                      for ct in (0, 1)]
                sss = list(range(min(NSS - 1, 4 * k + 4), (-1 if k == 0 else 4 * k), -1))
                ci = 0

                def drain_c(until_leq):
                    nonlocal ci
                    while ci < len(cs) and cs[ci][0] > until_leq:
                        emit_C(*cs[ci])
                        ci += 1

                for ss in sss:
                    drain_c(2 * ss - 2)
                    if ss == 4 * k + 4 and k < 3:
                        pass  # V transform pre-emitted by CD(k+1)
                    else:
                        emit_Dvt(ss)
                    emit_Dunit(ss, 0)
                    if ci < len(cs):
                        emit_C(*cs[ci])
                        ci += 1
                    emit_Dunit(ss, 1)
                    if ci < len(cs):
                        emit_C(*cs[ci])
                        ci += 1
                drain_c(-1)
                if k > 0:
                    emit_Dvt(4 * k)

            emit_deferred_consts()
            emit_group_A(3)
            for k in range(3, -1, -1):
                if k > 0:
                    emit_group_A(k - 1)
                emit_group_CD(k)
            if debug_taps:
                nc.sync.dma_start(dbg_pooled_d[:], pooled[:])
                for ct in range(2):
                    nc.sync.dma_start(dbg_out_d[ct], outb[ct][:])

    _legalize_waits(nc, mybir)
    return nc


def _fold_bn(w, bn):
    """BN(conv(x, w)) == conv(x, w * s[co]) + t[co]."""
    g, b, m, v = bn[0], bn[1], bn[2], bn[3]
    s = g / np.sqrt(v + EPS)
    t = b - m * s
    return w * s[:, None, None, None], t


def _prep_inputs(x, w_res, bn_res, w_vpre, bn_vpre, w_hpre, bn_hpre,
                 w_add, bn_add, w_post, bn_post):
    import ml_dtypes

    x = np.asarray(x, np.float32)
    xp = np.zeros((B, C, H, WP), np.float32)
    xp[:, :, :, 1 : 1 + W] = x
    x = xp.astype(ml_dtypes.bfloat16)
    w_res_s, t_res = _fold_bn(np.asarray(w_res, np.float32), np.asarray(bn_res, np.float32))
    w_vpre_s, t_vpre = _fold_bn(np.asarray(w_vpre, np.float32), np.asarray(bn_vpre, np.float32))
    w_hpre_s, t_hpre = _fold_bn(np.asarray(w_hpre, np.float32), np.asarray(bn_hpre, np.float32))
    w_add_s, t_add = _fold_bn(np.asarray(w_add, np.float32), np.asarray(bn_add, np.float32))
    w_post_s, t_post = _fold_bn(np.asarray(w_post, np.float32), np.asarray(bn_post, np.float32))

    # w_vh[k, kt*9+t, m]: m<64 vpre, m>=64 hpre; lhsT[k, m] = w[m, kt*128+k, dy, dx]
    w_vh = np.zeros((128, 18, 128), np.float32)
    for kt in range(2):
        for t, (dy, dx) in enumerate(TAPS):
            blk = kt * 128
            w_vh[:, kt * 9 + t, 0:64] = w_vpre_s[:, blk : blk + 128, dy, dx].T
            w_vh[:, kt * 9 + t, 64:128] = w_hpre_s[:, blk : blk + 128, dy, dx].T

    # w_res_l[k, ct*2+kt, m] = w_res_s[ct*128+m, kt*128+k]
    w_res_l = np.zeros((128, 4, 128), np.float32)
    for ct in range(2):
        for kt in range(2):
            w_res_l[:, ct * 2 + kt, :] = w_res_s[
                ct * 128 : (ct + 1) * 128, kt * 128 : (kt + 1) * 128, 0, 0
            ].T

    # w_mrg[k, ct*9+t, m] = w_add_s[ct*128+m, k%64, dy, dx]  (row-replicated)
    w_mrg = np.zeros((128, 18, 128), np.float32)
    for ct in range(2):
        for t, (dy, dx) in enumerate(TAPS):
            blkT = w_add_s[ct * 128 : (ct + 1) * 128, :, dy, dx].T  # [64, 128]
            w_mrg[0:64, ct * 9 + t, :] = blkT
            w_mrg[64:128, ct * 9 + t, :] = blkT

    # Winograd F(2,3) post-conv weights: for each (co, dy, point p, kt),
    # g0=w0, g1=(w0+w1+w2)/2, g2=(w0-w1+w2)/2, g3=w2 over the dx taps.
    # w_post_l[k, ((co*3+dy)*4+p)*2+kt, m] = g_p[co*128+m, kt*128+k]
    w_post_l = np.zeros((128, 48, 128), np.float32)
    for co in range(2):
        for dy in range(3):
            W3 = w_post_s[co * 128 : (co + 1) * 128, :, dy, :]  # [m, cin, dx]
            g = [W3[:, :, 0],
                 0.5 * (W3[:, :, 0] + W3[:, :, 1] + W3[:, :, 2]),
                 0.5 * (W3[:, :, 0] - W3[:, :, 1] + W3[:, :, 2]),
                 W3[:, :, 2]]
            for p in range(4):
                for kt in range(2):
                    idx = ((co * 3 + dy) * 4 + p) * 2 + kt
                    w_post_l[:, idx, :] = g[p][:, kt * 128 : (kt + 1) * 128].T

    biases = np.zeros((128, 5), np.float32)
    biases[0:64, 0] = t_vpre
    biases[64:128, 0] = t_hpre
    t_mrg = t_res + t_add
    biases[:, 1] = t_mrg[0:128]
    biases[:, 2] = t_mrg[128:256]
    biases[:, 3] = t_post[0:128]
    biases[:, 4] = t_post[128:256]

    shared = {
        "w_vh": w_vh.astype(ml_dtypes.bfloat16),
        "w_res_l": w_res_l.astype(ml_dtypes.bfloat16),
        "w_mrg": w_mrg.astype(ml_dtypes.bfloat16),
        "w_post_l": w_post_l.astype(ml_dtypes.bfloat16),
        "biases": biases,
    }
    return x, shared


def kernel(x, w_res, bn_res, w_vpre, bn_vpre, w_hpre, bn_hpre,
           w_add, bn_add, w_post, bn_post):
    from concourse.bass_utils import run_bass_kernel_spmd

    x, shared = _prep_inputs(x, w_res, bn_res, w_vpre, bn_vpre, w_hpre,
                             bn_hpre, w_add, bn_add, w_post, bn_post)

    if "nc" not in _CACHE:
        _CACHE["nc"] = build_nc()
    nc = _CACHE["nc"]

    in_maps = [dict(shared, x_s=np.ascontiguousarray(x[i])) for i in range(N_CORES)]
    res = run_bass_kernel_spmd(nc, in_maps, list(range(N_CORES)))
    return np.stack([res.results[i]["y"] for i in range(N_CORES)]).astype(np.float32)

